# revision 13
# baseline (speedup 1.0000x reference)
"""LIF spike (leaky integrate-and-fire) forward kernel for Trainium2.

Recurrence over the time axis T=8 of x[64,128,32,32,8] (fp32):
    u_t = TAU * u_{t-1} * (1 - o_{t-1}) + x_t
    o_t = (u_t > VTH)
Data-parallel over the batch dim: 8 NeuronCores x 8 batches each.

Layout: the host transposes each core's shard to time-major [T, spatial]
so that every time-step slice is a contiguous [128, FD] tile (unit-stride
APs for every engine op, contiguous >=1MiB DMAs). Per step the work is:
    o_t  = (u_t > VTH)                 DVE tensor_scalar is_gt -> fp32 out
    w_t  = TAU - TAU*o_t               ScalarE activation Copy(scale,bias),
                                       written in place over o_t after its
                                       store DMA has read it
    u_'  = u_t * w_t                   DVE tensor_tensor mult (in place)
    u_t1 = u_' + x_t1                  DVE tensor_tensor add (in place on
                                       the freshly loaded x tile)
The x tile doubles as the membrane-state buffer, the o tile doubles as the
w buffer, so SBUF holds just two fp32 pools.
"""

import sys

for _p in ("/opt/trn_rl_repo",):
    if _p not in sys.path:
        sys.path.insert(0, _p)

import numpy as np

TAU = 0.1
VTH = 1.5

B, C, H, W, T = 64, 128, 32, 32, 8
NCORES = 8
BS = B // NCORES                      # batches per core
SPAT = BS * C * H * W                 # spatial elems per core per step: 1,048,576
P = 128                               # partitions
FD = 2048                             # free dim per tile
NCH = SPAT // (P * FD)                # spatial chunks per step: 4
ROWS = T * NCH * P                    # dram rows (t-major): 4096
ELEMS = SPAT * T

_compiled = None


def _build_pk(spike: str = "act", xbufs: int = 12, obufs: int = 6, fd: int = FD):
    """Bit-packed output variant: one u8 byte per spatial element holding all
    T=8 spikes (bit t = o_t), cutting output HBM traffic 32x vs f32.

    Per time step t, per [P, fd] chunk c (engine assignment in parens):
      decay   u_t = TAU*u'_{t-1} + x_t        stt, in place on x tile  (Pool)
      spike   o_t = (u_t > VTH) as u8 {0,1}   (Act: Sign(u-VTH) -> u8, the
                                               -1 saturating to 0; or DVE/Pool
                                               tensor_scalar is_gt)
      pack    acc += o_t << t                 stt, acc is the u8 out tile (DVE)
      reset   u'_t = 0 where o_t              copy_predicated, mask=o_t (DVE)
    """
    import concourse.bacc as bacc
    import concourse.mybir as mybir
    import concourse.tile as tile

    nch = SPAT // (P * fd)
    nc = bacc.Bacc(
        "TRN2", target_bir_lowering=False, debug=False, num_devices=NCORES
    )
    f32 = mybir.dt.float32
    u8 = mybir.dt.uint8
    mult = mybir.AluOpType.mult
    add = mybir.AluOpType.add
    is_gt = mybir.AluOpType.is_gt
    is_le = mybir.AluOpType.is_le
    sign_f = mybir.ActivationFunctionType.Sign

    x_d = nc.dram_tensor("x", [T * nch * P, fd], f32, kind="ExternalInput").ap()
    o_d = nc.dram_tensor("o", [nch * P, fd], u8, kind="ExternalOutput").ap()

    with tile.TileContext(nc) as tc:
        with (
            tc.tile_pool(name="xp", bufs=xbufs) as xp,
            tc.tile_pool(name="op", bufs=obufs) as op_,
            tc.tile_pool(name="cp", bufs=1) as cp,
        ):
            zero = None
            nvth = None
            if spike != "ts":
                zero = cp.tile([P, fd], f32, tag="zero")
                nc.gpsimd.memset(zero[:], 0.0)
                nvth = cp.tile([P, 1], f32, tag="nvth")
                nc.gpsimd.memset(nvth[:], -VTH)
            # f32 accumulator (Pool can't do u8+u8 adds); u8 out tile is
            # written once by the final t=7 pack op.
            acc = [
                cp.tile([P, fd], f32, tag=f"acc{c}", name=f"acc{c}")
                for c in range(nch)
            ]
            out8 = [
                cp.tile([P, fd], u8, tag=f"out{c}", name=f"out{c}")
                for c in range(nch)
            ]
            st = [None] * nch
            for t in range(T):
                for c in range(nch):
                    r0 = (t * nch + c) * P
                    xt = xp.tile([P, fd], f32)
                    nc.sync.dma_start(out=xt[:], in_=x_d[r0 : r0 + P, :])
                    if t > 0:
                        # u_t = TAU*u' + x_t  (in place on the x tile).
                        # stt is DVE-only on v3 (Pool rejects TensorScalarPtr
                        # in the stt form).
                        nc.vector.scalar_tensor_tensor(
                            out=xt[:], in0=st[c][:], scalar=TAU, in1=xt[:],
                            op0=mult, op1=add,
                        )
                    st[c] = xt
                    if spike == "ts":
                        # mask-free: weighted spike + gated state, all-DVE
                        ws = op_.tile([P, fd], f32, name="wsf")
                        nc.vector.tensor_scalar(
                            ws[:], st[c][:], VTH, float(1 << t), is_gt, mult
                        )
                        if t == 0:
                            nc.vector.tensor_scalar(
                                acc[c][:], ws[:], 1.0, None, mult
                            )
                        else:
                            dst = out8[c] if t == T - 1 else acc[c]
                            nc.vector.tensor_tensor(
                                out=dst[:], in0=ws[:], in1=acc[c][:], op=add
                            )
                        if t < T - 1:
                            # u'' = (u <= VTH) * u   (kills spiked state)
                            nc.vector.scalar_tensor_tensor(
                                out=st[c][:], in0=st[c][:], scalar=VTH,
                                in1=st[c][:], op0=is_le, op1=mult,
                            )
                    else:
                        o = op_.tile([P, fd], u8)
                        if spike == "act":
                            # o = sign(u - VTH) -> u8: -1 saturates to 0
                            nc.scalar.activation(
                                o[:], st[c][:], sign_f, bias=nvth[:]
                            )
                        else:
                            eng = nc.vector if (t + c) % 2 else nc.gpsimd
                            eng.tensor_scalar(o[:], st[c][:], VTH, None, is_gt)
                        # pack: acc (f32) += o << t.  DVE already carries
                        # decay+reset (57us floor), so t<=5 pack goes to the
                        # otherwise-idle Pool engine (u8 ts, then the legal
                        # mixed u8+f32 tt add); t=6,7 are single DVE stt ops,
                        # t=7 writing the final u8 byte.
                        if t == 0:
                            nc.gpsimd.tensor_scalar(
                                acc[c][:], o[:], 1.0, None, mult
                            )
                        elif t <= 5:
                            ws = op_.tile([P, fd], u8, name="ws")
                            nc.gpsimd.tensor_scalar(
                                ws[:], o[:], float(1 << t), None, mult
                            )
                            nc.gpsimd.tensor_tensor(
                                out=acc[c][:], in0=ws[:], in1=acc[c][:],
                                op=add,
                            )
                        else:
                            dst = out8[c] if t == T - 1 else acc[c]
                            nc.vector.scalar_tensor_tensor(
                                out=dst[:], in0=o[:], scalar=float(1 << t),
                                in1=acc[c][:], op0=mult, op1=add,
                            )
                        if t < T - 1:
                            nc.vector.copy_predicated(
                                out=st[c][:], mask=o[:], data=zero[:]
                            )
                    if t == T - 1:
                        nc.sync.dma_start(
                            out=o_d[c * P : (c + 1) * P, :], in_=out8[c][:]
                        )
    nc.compile()
    return nc


def _build(
    reps: int = 1,
    mode: str = "full",
    bufs=(10, 10),
    fd=FD,
    odt: str = "f32",
    ger: str = "v",
    mer: str = "v",
    spike: str = "dve",
    pack: bool = False,
    ib: int = 2,
):
    import contextlib

    import concourse.bacc as bacc
    import concourse.mybir as mybir
    import concourse.tile as tile

    nch = SPAT // (P * fd)
    nc = bacc.Bacc(
        "TRN2",
        target_bir_lowering=False,
        debug=False,
        num_devices=NCORES,
    )
    f32 = mybir.dt.float32
    odtype = f32 if odt == "f32" else mybir.dt.int8
    if pack:
        # in rows (t, cg, p) cols (half, j); out rows (t, p) cols (c, j)
        x_d = nc.dram_tensor(
            "x", [T * (nch // ib) * P, ib * fd], f32, kind="ExternalInput"
        ).ap()
        o_d = nc.dram_tensor(
            "o", [T * P, nch * fd], mybir.dt.int8, kind="ExternalOutput"
        ).ap()
    else:
        x_d = nc.dram_tensor(
            "x", [T * nch * P, fd], f32, kind="ExternalInput"
        ).ap()
        o_d = nc.dram_tensor(
            "o", [T * nch * P, fd], odtype, kind="ExternalOutput"
        ).ap()

    with tile.TileContext(nc) as tc:
        with (
            tc.tile_pool(name="xp", bufs=bufs[0]) as xp,
            tc.tile_pool(name="op", bufs=bufs[1]) as op_,
            tc.tile_pool(name="wp", bufs=6) as wp,
            tc.tile_pool(name="cp", bufs=1) as cp,
        ):
            rep_ctx = (
                tc.For_i(0, reps, 1) if reps > 1 else contextlib.nullcontext()
            )
            with rep_ctx:
                if pack:
                    _emit_packed(nc, xp, op_, cp, x_d, o_d, mybir, mode,
                                 fd, nch, ib)
                else:
                    _emit(nc, xp, op_, wp, cp, x_d, o_d, mybir, mode, fd,
                          nch, odt, ger, mer, spike)
    nc.compile()
    return nc


def _emit_packed(nc, xp, op_, cp, x_d, o_d, mybir, mode, fd, nch, ib):
    """act1-spike i8-out variant with batched DMAs.

    Input tiles span `ib` chunks (one contiguous DMA each); output tiles
    span all `nch` chunks of a step (one contiguous DMA per step).
    """
    f32 = mybir.dt.float32
    i8 = mybir.dt.int8
    mult = mybir.AluOpType.mult
    add = mybir.AluOpType.add
    relu_f = mybir.ActivationFunctionType.Relu
    dma, compute = mode in ("full", "dma"), mode in ("full", "compute")
    ng = nch // ib

    zero = cp.tile([P, fd], f32, tag="zero")
    nc.gpsimd.memset(zero[:], 0.0)
    nvthbig = cp.tile([P, 1], f32, tag="nvthbig")
    nc.gpsimd.memset(nvthbig[:], -VTH * 1e9)

    u = [None] * nch       # AP slice holding u_t per chunk
    o_prev = [None] * nch  # AP slice of o_{t-1} per chunk
    for t in range(T):
        xts = []
        for g in range(ng):
            xt = xp.tile([P, ib * fd], f32)
            if dma:
                r0 = (t * ng + g) * P
                nc.sync.dma_start(out=xt[:], in_=x_d[r0 : r0 + P, :])
            elif t == 0:
                nc.gpsimd.memset(xt[:], 0.25)
            xts.append(xt)
        ot = op_.tile([P, nch * fd], i8)
        for c in range(nch):
            g, h = c // ib, c % ib
            xs = xts[g][:, h * fd : (h + 1) * fd]
            if compute:
                if t > 0:
                    # reset where previous step spiked
                    nc.vector.copy_predicated(
                        out=u[c], mask=o_prev[c], data=zero[:]
                    )
                    # u_t = TAU*u_masked + x_t  (in place on x slice)
                    nc.vector.scalar_tensor_tensor(
                        out=xs, in0=u[c], scalar=TAU, in1=xs,
                        op0=mult, op1=add,
                    )
                u[c] = xs
                # o8 = sat_i8(relu(1e9*u - 1e9*VTH)): nonzero iff spike
                nc.scalar.activation(
                    ot[:, c * fd : (c + 1) * fd], u[c], relu_f,
                    bias=nvthbig[:], scale=1e9,
                )
                o_prev[c] = ot[:, c * fd : (c + 1) * fd]
        if not compute:
            nc.gpsimd.memset(ot[:, :1], 1)
        if dma:
            nc.sync.dma_start(out=o_d[t * P : (t + 1) * P, :], in_=ot[:])


def _emit(nc, xp, op_, wp, cp, x_d, o_d, mybir, mode, fd, nch, odt, ger, mer,
          spike="dve"):
    f32 = mybir.dt.float32
    mult = mybir.AluOpType.mult
    add = mybir.AluOpType.add
    is_gt = mybir.AluOpType.is_gt
    copy_f = mybir.ActivationFunctionType.Copy
    dma, compute = mode in ("full", "dma"), mode in ("full", "compute")
    odtype = f32 if odt == "f32" else mybir.dt.int8
    geng = nc.vector if ger == "v" else nc.gpsimd
    meng = nc.vector if mer == "v" else nc.gpsimd

    i8 = mybir.dt.int8
    relu_f = mybir.ActivationFunctionType.Relu
    sign_f = mybir.ActivationFunctionType.Sign

    o8c = None
    if mode == "dma" and odt == "i8":
        o8c = cp.tile([P, fd], i8, tag="o8c")
        nc.gpsimd.memset(o8c[:], 1)
    if spike in ("act", "act1"):
        assert odt == "i8"
        zero = cp.tile([P, fd], f32, tag="zero")
        nc.gpsimd.memset(zero[:], 0.0)
        nvth = cp.tile([P, 1], f32, tag="nvth")
        nc.gpsimd.memset(nvth[:], -VTH)
        nvthbig = cp.tile([P, 1], f32, tag="nvthbig")
        nc.gpsimd.memset(nvthbig[:], -VTH * 1e9)

    u = [None] * nch       # tile holding u_t per chunk
    o_prev = [None] * nch  # tile holding o_{t-1} per chunk
    for t in range(T):
        for c in range(nch):
            r0 = (t * nch + c) * P
            xt = xp.tile([P, fd], f32)
            if dma:
                nc.sync.dma_start(out=xt[:], in_=x_d[r0 : r0 + P, :])
            elif t == 0:
                nc.gpsimd.memset(xt[:], 0.25)
            if compute and spike in ("act", "act1"):
                if t > 0:
                    o = o_prev[c]
                    # reset where previous step spiked
                    nc.vector.copy_predicated(
                        out=u[c][:], mask=o[:], data=zero[:]
                    )
                    # u_t = TAU*u_masked + x_t  (in place on x tile)
                    nc.vector.scalar_tensor_tensor(
                        out=xt[:], in0=u[c][:], scalar=TAU, in1=xt[:],
                        op0=mult, op1=add,
                    )
                u[c] = xt
                ot = op_.tile([P, fd], i8)
                if spike == "act1":
                    # o8 = sat_i8(relu(1e9*u - 1e9*VTH)): nonzero iff spike.
                    # int8 conversion saturates at 127 (verified on HW), and
                    # |u-VTH| >= 1 ulp(1.5) so the *1e9 rounding never
                    # crosses zero.
                    nc.scalar.activation(
                        ot[:], u[c][:], relu_f, bias=nvthbig[:], scale=1e9
                    )
                else:
                    # spike on ScalarE: sg = sign(u - VTH); o = relu(sg)
                    sg = wp.tile([P, fd], f32, tag="sg")
                    nc.scalar.activation(sg[:], u[c][:], sign_f, bias=nvth[:])
                    nc.scalar.activation(ot[:], sg[:], relu_f)
                o_prev[c] = ot
            elif compute:
                if t == 0:
                    u[c] = xt
                else:
                    o = o_prev[c]
                    if odt == "f32":
                        # w <- TAU - TAU*o  (in place over o after its store)
                        w = o
                        nc.scalar.activation(
                            w[:], o[:], copy_f, bias=TAU, scale=-TAU
                        )
                    else:
                        w = wp.tile([P, fd], f32)
                        nc.scalar.activation(
                            w[:], o[:], copy_f, bias=TAU, scale=-TAU
                        )
                    # u_masked = u_{t-1} * w   (in place)
                    meng.tensor_tensor(
                        out=u[c][:], in0=u[c][:], in1=w[:], op=mult
                    )
                    # u_t = u_masked + x_t    (in place on x tile)
                    nc.vector.tensor_tensor(
                        out=xt[:], in0=u[c][:], in1=xt[:], op=add
                    )
                    u[c] = xt
                ot = op_.tile([P, fd], odtype)
                geng.tensor_scalar(ot[:], u[c][:], VTH, None, is_gt)
                o_prev[c] = ot
            else:
                ot = o8c if o8c is not None else xt
            if dma:
                nc.sync.dma_start(out=o_d[r0 : r0 + P, :], in_=ot[:])


def _get_compiled():
    global _compiled
    if _compiled is None:
        import os

        mode = os.environ.get("LIF_MODE", "pk")
        if mode == "pk":
            _compiled = _build_pk(
                spike=os.environ.get("LIF_SPIKE", "act"),
                xbufs=int(os.environ.get("LIF_XBUFS", "12")),
                obufs=int(os.environ.get("LIF_OBUFS", "6")),
            )
        else:
            _compiled = _build()
    return _compiled


def _shard_tmajor(x: np.ndarray, i: int) -> np.ndarray:
    """Core i's shard, time-major: [T*NCH*P, FD], row-major over (t, spatial)."""
    xs = x[i * BS : (i + 1) * BS]                   # [BS,C,H,W,T]
    xt = np.moveaxis(xs.reshape(SPAT, T), -1, 0)    # [T, SPAT]
    return np.ascontiguousarray(xt).reshape(ROWS, FD)


def kernel(x: np.ndarray, _trace: bool = False):
    nc = _get_compiled()
    from concourse.bass_utils import run_bass_kernel_spmd

    x = np.asarray(x, dtype=np.float32)
    in_maps = [{"x": _shard_tmajor(x, i)} for i in range(NCORES)]
    res = run_bass_kernel_spmd(
        nc, in_maps, core_ids=list(range(NCORES)), trace=_trace
    )
    outs = []
    for r in res.results:
        ot = r["o"]
        if ot.size == SPAT:                         # bit-packed u8: bit t = o_t
            bits = np.unpackbits(
                ot.reshape(-1, 1), axis=1, bitorder="little"
            )[:, :T]
            outs.append(bits.reshape(BS, C, H, W, T).astype(np.float32))
            continue
        if ot.dtype != np.float32:                  # int8 spikes -> f32
            ot = (ot != 0).astype(np.float32)
        ot = ot.reshape(T, SPAT)                    # time-major back to T-last
        outs.append(np.moveaxis(ot, 0, -1).reshape(BS, C, H, W, T))
    out = np.ascontiguousarray(np.concatenate(outs, axis=0))
    return (out, res) if _trace else out



# revision 19
# speedup vs baseline: 4.2575x; 4.2575x over previous
"""LIF spike (leaky integrate-and-fire) forward kernel for Trainium2.

Recurrence over the time axis T=8 of x[64,128,32,32,8] (fp32):
    u_t = TAU * u_{t-1} * (1 - o_{t-1}) + x_t
    o_t = (u_t > VTH)
Data-parallel over the batch dim: 8 NeuronCores x 8 batches each.

Layout: the host transposes each core's shard to time-major [T, spatial]
so that every time-step slice is a contiguous [128, FD] tile (unit-stride
APs for every engine op, contiguous >=1MiB DMAs). Per step the work is:
    o_t  = (u_t > VTH)                 DVE tensor_scalar is_gt -> fp32 out
    w_t  = TAU - TAU*o_t               ScalarE activation Copy(scale,bias),
                                       written in place over o_t after its
                                       store DMA has read it
    u_'  = u_t * w_t                   DVE tensor_tensor mult (in place)
    u_t1 = u_' + x_t1                  DVE tensor_tensor add (in place on
                                       the freshly loaded x tile)
The x tile doubles as the membrane-state buffer, the o tile doubles as the
w buffer, so SBUF holds just two fp32 pools.
"""

import sys

for _p in ("/opt/trn_rl_repo",):
    if _p not in sys.path:
        sys.path.insert(0, _p)

import numpy as np

TAU = 0.1
VTH = 1.5

B, C, H, W, T = 64, 128, 32, 32, 8
NCORES = 8
BS = B // NCORES                      # batches per core
SPAT = BS * C * H * W                 # spatial elems per core per step: 1,048,576
P = 128                               # partitions
FD = 2048                             # free dim per tile
NCH = SPAT // (P * FD)                # spatial chunks per step: 4
ROWS = T * NCH * P                    # dram rows (t-major): 4096
ELEMS = SPAT * T

_compiled = None

# v-domain scaling: v_t = 10^t * u_t kills the TAU multiply (host pre-scales
# x_t by 10^t); thresholds 1.5*10^t are all exact in f32.
VSCALE = [float(10.0**t) for t in range(T)]
VTH_T = [float(1.5 * 10.0**t) for t in range(T)]


def _build_pe(fd: int = 1024, kadd: int = 6, meng: str = "pool",
              xbufs: int = 16, mbufs: int = 8, pbufs: int = 4):
    """v-domain LIF with PE-packed output bytes.

    Recurrence per chunk c (sequential in t):
        v_t = v_{t-1} * m_{t-1} + xs_t      xs_t = 10^t * x_t (host-scaled)
        m_t = (v_t <= 1.5*10^t)             keep-mask, bf16 {0,1}  (DVE ts)
    Packing on the otherwise-idle PE: psum += (2^t I) @ m_t over the 8 steps
    gives byte = sum_t m_t 2^t (exact: bf16 holds {0,1} and 2^t; PSUM is
    f32).  Act copies PSUM -> SBUF u8; host inverts bits (o = NOT m).
    The reset multiply runs on Pool (pure-ish tt), adds split DVE/Pool via
    `kadd` (# adds per chunk on DVE).
    """
    import concourse.bacc as bacc
    import concourse.mybir as mybir
    import concourse.tile as tile

    nch = SPAT // (P * fd)
    nc = bacc.Bacc(
        "TRN2", target_bir_lowering=False, debug=False, num_devices=NCORES
    )
    f32 = mybir.dt.float32
    bf16 = mybir.dt.bfloat16
    u8 = mybir.dt.uint8
    mult = mybir.AluOpType.mult
    add = mybir.AluOpType.add
    is_le = mybir.AluOpType.is_le
    copy_f = mybir.ActivationFunctionType.Copy

    x_d = nc.dram_tensor(
        "x", [nch * T * P, fd], f32, kind="ExternalInput"
    ).ap()
    w_d = nc.dram_tensor("w", [T * P, P], bf16, kind="ExternalInput").ap()
    o_d = nc.dram_tensor("o", [nch * P, fd], u8, kind="ExternalOutput").ap()

    with tile.TileContext(nc) as tc:
        with (
            tc.tile_pool(name="xp", bufs=xbufs) as xp,
            tc.tile_pool(name="mp", bufs=mbufs) as mp,
            tc.tile_pool(name="op", bufs=2) as op_,
            tc.tile_pool(name="wp", bufs=1) as wp,
            tc.psum_pool(name="pp", bufs=pbufs) as pp,
        ):
            wts = []
            for t in range(T):
                wt = wp.tile([P, P], bf16, tag=f"w{t}", name=f"w{t}")
                nc.sync.dma_start(out=wt[:], in_=w_d[t * P : (t + 1) * P, :])
                wts.append(wt)
            for c in range(nch):
                ps = pp.tile([P, fd], f32, name="ps")
                st = None
                mprev = None
                for t in range(T):
                    r0 = (c * T + t) * P
                    xt = xp.tile([P, fd], f32)
                    nc.sync.dma_start(out=xt[:], in_=x_d[r0 : r0 + P, :])
                    if t > 0:
                        # um = v_{t-1} * m_{t-1}  (in place on state tile)
                        me = nc.gpsimd if meng == "pool" else nc.vector
                        me.tensor_tensor(
                            out=st[:], in0=st[:], in1=mprev[:], op=mult
                        )
                        # v_t = um + xs_t  (in place on the x tile)
                        ae = nc.vector if t <= kadd else nc.gpsimd
                        ae.tensor_tensor(
                            out=xt[:], in0=st[:], in1=xt[:], op=add
                        )
                    st = xt
                    m = mp.tile([P, fd], bf16)
                    nc.vector.tensor_scalar(
                        m[:], st[:], VTH_T[t], None, is_le
                    )
                    # PSUM bank limit: <=512 f32 out columns per matmul
                    for h in range(fd // 512):
                        sl = slice(h * 512, (h + 1) * 512)
                        nc.tensor.matmul(
                            ps[:, sl], wts[t][:], m[:, sl],
                            start=(t == 0), stop=(t == T - 1),
                        )
                    mprev = m
                ot = op_.tile([P, fd], u8)
                nc.scalar.activation(ot[:], ps[:], copy_f)
                nc.sync.dma_start(
                    out=o_d[c * P : (c + 1) * P, :], in_=ot[:]
                )
    nc.compile()
    return nc


def _build_pk(spike: str = "act", xbufs: int = 12, obufs: int = 6, fd: int = FD):
    """Bit-packed output variant: one u8 byte per spatial element holding all
    T=8 spikes (bit t = o_t), cutting output HBM traffic 32x vs f32.

    Per time step t, per [P, fd] chunk c (engine assignment in parens):
      decay   u_t = TAU*u'_{t-1} + x_t        stt, in place on x tile  (Pool)
      spike   o_t = (u_t > VTH) as u8 {0,1}   (Act: Sign(u-VTH) -> u8, the
                                               -1 saturating to 0; or DVE/Pool
                                               tensor_scalar is_gt)
      pack    acc += o_t << t                 stt, acc is the u8 out tile (DVE)
      reset   u'_t = 0 where o_t              copy_predicated, mask=o_t (DVE)
    """
    import concourse.bacc as bacc
    import concourse.mybir as mybir
    import concourse.tile as tile

    nch = SPAT // (P * fd)
    nc = bacc.Bacc(
        "TRN2", target_bir_lowering=False, debug=False, num_devices=NCORES
    )
    f32 = mybir.dt.float32
    u8 = mybir.dt.uint8
    mult = mybir.AluOpType.mult
    add = mybir.AluOpType.add
    is_gt = mybir.AluOpType.is_gt
    is_le = mybir.AluOpType.is_le
    sign_f = mybir.ActivationFunctionType.Sign

    x_d = nc.dram_tensor("x", [T * nch * P, fd], f32, kind="ExternalInput").ap()
    o_d = nc.dram_tensor("o", [nch * P, fd], u8, kind="ExternalOutput").ap()

    with tile.TileContext(nc) as tc:
        with (
            tc.tile_pool(name="xp", bufs=xbufs) as xp,
            tc.tile_pool(name="op", bufs=obufs) as op_,
            tc.tile_pool(name="cp", bufs=1) as cp,
        ):
            zero = None
            nvth = None
            if spike != "ts":
                zero = cp.tile([P, fd], f32, tag="zero")
                nc.gpsimd.memset(zero[:], 0.0)
                nvth = cp.tile([P, 1], f32, tag="nvth")
                nc.gpsimd.memset(nvth[:], -VTH)
            # f32 accumulator (Pool can't do u8+u8 adds); u8 out tile is
            # written once by the final t=7 pack op.
            acc = [
                cp.tile([P, fd], f32, tag=f"acc{c}", name=f"acc{c}")
                for c in range(nch)
            ]
            out8 = [
                cp.tile([P, fd], u8, tag=f"out{c}", name=f"out{c}")
                for c in range(nch)
            ]
            st = [None] * nch
            for t in range(T):
                for c in range(nch):
                    r0 = (t * nch + c) * P
                    xt = xp.tile([P, fd], f32)
                    nc.sync.dma_start(out=xt[:], in_=x_d[r0 : r0 + P, :])
                    if t > 0:
                        # u_t = TAU*u' + x_t  (in place on the x tile).
                        # stt is DVE-only on v3 (Pool rejects TensorScalarPtr
                        # in the stt form).
                        nc.vector.scalar_tensor_tensor(
                            out=xt[:], in0=st[c][:], scalar=TAU, in1=xt[:],
                            op0=mult, op1=add,
                        )
                    st[c] = xt
                    if spike == "ts":
                        # mask-free: weighted spike + gated state, all-DVE
                        ws = op_.tile([P, fd], f32, name="wsf")
                        nc.vector.tensor_scalar(
                            ws[:], st[c][:], VTH, float(1 << t), is_gt, mult
                        )
                        if t == 0:
                            nc.vector.tensor_scalar(
                                acc[c][:], ws[:], 1.0, None, mult
                            )
                        else:
                            dst = out8[c] if t == T - 1 else acc[c]
                            nc.vector.tensor_tensor(
                                out=dst[:], in0=ws[:], in1=acc[c][:], op=add
                            )
                        if t < T - 1:
                            # u'' = (u <= VTH) * u   (kills spiked state)
                            nc.vector.scalar_tensor_tensor(
                                out=st[c][:], in0=st[c][:], scalar=VTH,
                                in1=st[c][:], op0=is_le, op1=mult,
                            )
                    else:
                        o = op_.tile([P, fd], u8)
                        if spike == "act":
                            # o = sign(u - VTH) -> u8: -1 saturates to 0
                            nc.scalar.activation(
                                o[:], st[c][:], sign_f, bias=nvth[:]
                            )
                        else:
                            eng = nc.vector if (t + c) % 2 else nc.gpsimd
                            eng.tensor_scalar(o[:], st[c][:], VTH, None, is_gt)
                        # pack: acc (f32) += o << t.  DVE already carries
                        # decay+reset (57us floor), so t<=5 pack goes to the
                        # otherwise-idle Pool engine (u8 ts, then the legal
                        # mixed u8+f32 tt add); t=6,7 are single DVE stt ops,
                        # t=7 writing the final u8 byte.
                        if t == 0:
                            nc.gpsimd.tensor_scalar(
                                acc[c][:], o[:], 1.0, None, mult
                            )
                        elif t <= 5:
                            ws = op_.tile([P, fd], u8, name="ws")
                            nc.gpsimd.tensor_scalar(
                                ws[:], o[:], float(1 << t), None, mult
                            )
                            nc.gpsimd.tensor_tensor(
                                out=acc[c][:], in0=ws[:], in1=acc[c][:],
                                op=add,
                            )
                        else:
                            dst = out8[c] if t == T - 1 else acc[c]
                            nc.vector.scalar_tensor_tensor(
                                out=dst[:], in0=o[:], scalar=float(1 << t),
                                in1=acc[c][:], op0=mult, op1=add,
                            )
                        if t < T - 1:
                            nc.vector.copy_predicated(
                                out=st[c][:], mask=o[:], data=zero[:]
                            )
                    if t == T - 1:
                        nc.sync.dma_start(
                            out=o_d[c * P : (c + 1) * P, :], in_=out8[c][:]
                        )
    nc.compile()
    return nc


def _build(
    reps: int = 1,
    mode: str = "full",
    bufs=(10, 10),
    fd=FD,
    odt: str = "f32",
    ger: str = "v",
    mer: str = "v",
    spike: str = "dve",
    pack: bool = False,
    ib: int = 2,
):
    import contextlib

    import concourse.bacc as bacc
    import concourse.mybir as mybir
    import concourse.tile as tile

    nch = SPAT // (P * fd)
    nc = bacc.Bacc(
        "TRN2",
        target_bir_lowering=False,
        debug=False,
        num_devices=NCORES,
    )
    f32 = mybir.dt.float32
    odtype = f32 if odt == "f32" else mybir.dt.int8
    if pack:
        # in rows (t, cg, p) cols (half, j); out rows (t, p) cols (c, j)
        x_d = nc.dram_tensor(
            "x", [T * (nch // ib) * P, ib * fd], f32, kind="ExternalInput"
        ).ap()
        o_d = nc.dram_tensor(
            "o", [T * P, nch * fd], mybir.dt.int8, kind="ExternalOutput"
        ).ap()
    else:
        x_d = nc.dram_tensor(
            "x", [T * nch * P, fd], f32, kind="ExternalInput"
        ).ap()
        o_d = nc.dram_tensor(
            "o", [T * nch * P, fd], odtype, kind="ExternalOutput"
        ).ap()

    with tile.TileContext(nc) as tc:
        with (
            tc.tile_pool(name="xp", bufs=bufs[0]) as xp,
            tc.tile_pool(name="op", bufs=bufs[1]) as op_,
            tc.tile_pool(name="wp", bufs=6) as wp,
            tc.tile_pool(name="cp", bufs=1) as cp,
        ):
            rep_ctx = (
                tc.For_i(0, reps, 1) if reps > 1 else contextlib.nullcontext()
            )
            with rep_ctx:
                if pack:
                    _emit_packed(nc, xp, op_, cp, x_d, o_d, mybir, mode,
                                 fd, nch, ib)
                else:
                    _emit(nc, xp, op_, wp, cp, x_d, o_d, mybir, mode, fd,
                          nch, odt, ger, mer, spike)
    nc.compile()
    return nc


def _emit_packed(nc, xp, op_, cp, x_d, o_d, mybir, mode, fd, nch, ib):
    """act1-spike i8-out variant with batched DMAs.

    Input tiles span `ib` chunks (one contiguous DMA each); output tiles
    span all `nch` chunks of a step (one contiguous DMA per step).
    """
    f32 = mybir.dt.float32
    i8 = mybir.dt.int8
    mult = mybir.AluOpType.mult
    add = mybir.AluOpType.add
    relu_f = mybir.ActivationFunctionType.Relu
    dma, compute = mode in ("full", "dma"), mode in ("full", "compute")
    ng = nch // ib

    zero = cp.tile([P, fd], f32, tag="zero")
    nc.gpsimd.memset(zero[:], 0.0)
    nvthbig = cp.tile([P, 1], f32, tag="nvthbig")
    nc.gpsimd.memset(nvthbig[:], -VTH * 1e9)

    u = [None] * nch       # AP slice holding u_t per chunk
    o_prev = [None] * nch  # AP slice of o_{t-1} per chunk
    for t in range(T):
        xts = []
        for g in range(ng):
            xt = xp.tile([P, ib * fd], f32)
            if dma:
                r0 = (t * ng + g) * P
                nc.sync.dma_start(out=xt[:], in_=x_d[r0 : r0 + P, :])
            elif t == 0:
                nc.gpsimd.memset(xt[:], 0.25)
            xts.append(xt)
        ot = op_.tile([P, nch * fd], i8)
        for c in range(nch):
            g, h = c // ib, c % ib
            xs = xts[g][:, h * fd : (h + 1) * fd]
            if compute:
                if t > 0:
                    # reset where previous step spiked
                    nc.vector.copy_predicated(
                        out=u[c], mask=o_prev[c], data=zero[:]
                    )
                    # u_t = TAU*u_masked + x_t  (in place on x slice)
                    nc.vector.scalar_tensor_tensor(
                        out=xs, in0=u[c], scalar=TAU, in1=xs,
                        op0=mult, op1=add,
                    )
                u[c] = xs
                # o8 = sat_i8(relu(1e9*u - 1e9*VTH)): nonzero iff spike
                nc.scalar.activation(
                    ot[:, c * fd : (c + 1) * fd], u[c], relu_f,
                    bias=nvthbig[:], scale=1e9,
                )
                o_prev[c] = ot[:, c * fd : (c + 1) * fd]
        if not compute:
            nc.gpsimd.memset(ot[:, :1], 1)
        if dma:
            nc.sync.dma_start(out=o_d[t * P : (t + 1) * P, :], in_=ot[:])


def _emit(nc, xp, op_, wp, cp, x_d, o_d, mybir, mode, fd, nch, odt, ger, mer,
          spike="dve"):
    f32 = mybir.dt.float32
    mult = mybir.AluOpType.mult
    add = mybir.AluOpType.add
    is_gt = mybir.AluOpType.is_gt
    copy_f = mybir.ActivationFunctionType.Copy
    dma, compute = mode in ("full", "dma"), mode in ("full", "compute")
    odtype = f32 if odt == "f32" else mybir.dt.int8
    geng = nc.vector if ger == "v" else nc.gpsimd
    meng = nc.vector if mer == "v" else nc.gpsimd

    i8 = mybir.dt.int8
    relu_f = mybir.ActivationFunctionType.Relu
    sign_f = mybir.ActivationFunctionType.Sign

    o8c = None
    if mode == "dma" and odt == "i8":
        o8c = cp.tile([P, fd], i8, tag="o8c")
        nc.gpsimd.memset(o8c[:], 1)
    if spike in ("act", "act1"):
        assert odt == "i8"
        zero = cp.tile([P, fd], f32, tag="zero")
        nc.gpsimd.memset(zero[:], 0.0)
        nvth = cp.tile([P, 1], f32, tag="nvth")
        nc.gpsimd.memset(nvth[:], -VTH)
        nvthbig = cp.tile([P, 1], f32, tag="nvthbig")
        nc.gpsimd.memset(nvthbig[:], -VTH * 1e9)

    u = [None] * nch       # tile holding u_t per chunk
    o_prev = [None] * nch  # tile holding o_{t-1} per chunk
    for t in range(T):
        for c in range(nch):
            r0 = (t * nch + c) * P
            xt = xp.tile([P, fd], f32)
            if dma:
                nc.sync.dma_start(out=xt[:], in_=x_d[r0 : r0 + P, :])
            elif t == 0:
                nc.gpsimd.memset(xt[:], 0.25)
            if compute and spike in ("act", "act1"):
                if t > 0:
                    o = o_prev[c]
                    # reset where previous step spiked
                    nc.vector.copy_predicated(
                        out=u[c][:], mask=o[:], data=zero[:]
                    )
                    # u_t = TAU*u_masked + x_t  (in place on x tile)
                    nc.vector.scalar_tensor_tensor(
                        out=xt[:], in0=u[c][:], scalar=TAU, in1=xt[:],
                        op0=mult, op1=add,
                    )
                u[c] = xt
                ot = op_.tile([P, fd], i8)
                if spike == "act1":
                    # o8 = sat_i8(relu(1e9*u - 1e9*VTH)): nonzero iff spike.
                    # int8 conversion saturates at 127 (verified on HW), and
                    # |u-VTH| >= 1 ulp(1.5) so the *1e9 rounding never
                    # crosses zero.
                    nc.scalar.activation(
                        ot[:], u[c][:], relu_f, bias=nvthbig[:], scale=1e9
                    )
                else:
                    # spike on ScalarE: sg = sign(u - VTH); o = relu(sg)
                    sg = wp.tile([P, fd], f32, tag="sg")
                    nc.scalar.activation(sg[:], u[c][:], sign_f, bias=nvth[:])
                    nc.scalar.activation(ot[:], sg[:], relu_f)
                o_prev[c] = ot
            elif compute:
                if t == 0:
                    u[c] = xt
                else:
                    o = o_prev[c]
                    if odt == "f32":
                        # w <- TAU - TAU*o  (in place over o after its store)
                        w = o
                        nc.scalar.activation(
                            w[:], o[:], copy_f, bias=TAU, scale=-TAU
                        )
                    else:
                        w = wp.tile([P, fd], f32)
                        nc.scalar.activation(
                            w[:], o[:], copy_f, bias=TAU, scale=-TAU
                        )
                    # u_masked = u_{t-1} * w   (in place)
                    meng.tensor_tensor(
                        out=u[c][:], in0=u[c][:], in1=w[:], op=mult
                    )
                    # u_t = u_masked + x_t    (in place on x tile)
                    nc.vector.tensor_tensor(
                        out=xt[:], in0=u[c][:], in1=xt[:], op=add
                    )
                    u[c] = xt
                ot = op_.tile([P, fd], odtype)
                geng.tensor_scalar(ot[:], u[c][:], VTH, None, is_gt)
                o_prev[c] = ot
            else:
                ot = o8c if o8c is not None else xt
            if dma:
                nc.sync.dma_start(out=o_d[r0 : r0 + P, :], in_=ot[:])


def _mode():
    import os

    return os.environ.get("LIF_MODE", "pe")


def _get_compiled():
    global _compiled
    if _compiled is None:
        import os

        mode = _mode()
        if mode == "pe":
            _compiled = _build_pe(
                fd=int(os.environ.get("LIF_FD", "1024")),
                kadd=int(os.environ.get("LIF_KADD", "6")),
                meng=os.environ.get("LIF_MENG", "pool"),
                xbufs=int(os.environ.get("LIF_XBUFS", "16")),
                pbufs=int(os.environ.get("LIF_PBUFS", "4")),
            )
        elif mode == "pk":
            _compiled = _build_pk(
                spike=os.environ.get("LIF_SPIKE", "act"),
                xbufs=int(os.environ.get("LIF_XBUFS", "12")),
                obufs=int(os.environ.get("LIF_OBUFS", "6")),
            )
        elif mode == "act1":
            _compiled = _build(spike="act1", odt="i8")
        else:
            _compiled = _build()
    return _compiled


def _shard_pe(x: np.ndarray, i: int, fd: int) -> np.ndarray:
    """Core i's shard for pe mode: chunk-major [(c,t,p), fd] rows, with
    x_t pre-scaled by 10^t (v-domain)."""
    nch = SPAT // (P * fd)
    xs = x[i * BS : (i + 1) * BS].reshape(SPAT, T)
    xv = xs * np.asarray(VSCALE, dtype=np.float32)[None, :]
    xv = xv.reshape(nch, P, fd, T).transpose(0, 3, 1, 2)  # [c, t, P, fd]
    return np.ascontiguousarray(xv).reshape(nch * T * P, fd)


def _w_pe() -> np.ndarray:
    import ml_dtypes

    eye = np.eye(P, dtype=np.float32)
    w = np.concatenate([eye * float(1 << t) for t in range(T)], axis=0)
    return w.astype(ml_dtypes.bfloat16)


def _shard_tmajor(x: np.ndarray, i: int) -> np.ndarray:
    """Core i's shard, time-major: [T*NCH*P, FD], row-major over (t, spatial)."""
    xs = x[i * BS : (i + 1) * BS]                   # [BS,C,H,W,T]
    xt = np.moveaxis(xs.reshape(SPAT, T), -1, 0)    # [T, SPAT]
    return np.ascontiguousarray(xt).reshape(ROWS, FD)


def kernel(x: np.ndarray, _trace: bool = False):
    nc = _get_compiled()
    from concourse.bass_utils import run_bass_kernel_spmd

    x = np.asarray(x, dtype=np.float32)
    if _mode() == "pe":
        import os

        fd = int(os.environ.get("LIF_FD", "1024"))
        w = _w_pe()
        in_maps = [
            {"x": _shard_pe(x, i, fd), "w": w} for i in range(NCORES)
        ]
    else:
        in_maps = [{"x": _shard_tmajor(x, i)} for i in range(NCORES)]
    res = run_bass_kernel_spmd(
        nc, in_maps, core_ids=list(range(NCORES)), trace=_trace
    )
    invert = _mode() == "pe"                        # pe packs the keep-mask
    outs = []
    for r in res.results:
        ot = r["o"]
        if ot.size == SPAT:                         # bit-packed u8: bit t = o_t
            if invert:
                ot = np.invert(ot)
            bits = np.unpackbits(
                ot.reshape(-1, 1), axis=1, bitorder="little"
            )[:, :T]
            outs.append(bits.reshape(BS, C, H, W, T).astype(np.float32))
            continue
        if ot.dtype != np.float32:                  # int8 spikes -> f32
            ot = (ot != 0).astype(np.float32)
        ot = ot.reshape(T, SPAT)                    # time-major back to T-last
        outs.append(np.moveaxis(ot, 0, -1).reshape(BS, C, H, W, T))
    out = np.ascontiguousarray(np.concatenate(outs, axis=0))
    return (out, res) if _trace else out



# revision 22
# speedup vs baseline: 5.2584x; 1.2351x over previous
"""LIF spike (leaky integrate-and-fire) forward kernel for Trainium2.

Recurrence over the time axis T=8 of x[64,128,32,32,8] (fp32):
    u_t = TAU * u_{t-1} * (1 - o_{t-1}) + x_t
    o_t = (u_t > VTH)
Data-parallel over the batch dim: 8 NeuronCores x 8 batches each.

Layout: the host transposes each core's shard to time-major [T, spatial]
so that every time-step slice is a contiguous [128, FD] tile (unit-stride
APs for every engine op, contiguous >=1MiB DMAs). Per step the work is:
    o_t  = (u_t > VTH)                 DVE tensor_scalar is_gt -> fp32 out
    w_t  = TAU - TAU*o_t               ScalarE activation Copy(scale,bias),
                                       written in place over o_t after its
                                       store DMA has read it
    u_'  = u_t * w_t                   DVE tensor_tensor mult (in place)
    u_t1 = u_' + x_t1                  DVE tensor_tensor add (in place on
                                       the freshly loaded x tile)
The x tile doubles as the membrane-state buffer, the o tile doubles as the
w buffer, so SBUF holds just two fp32 pools.
"""

import sys

for _p in ("/opt/trn_rl_repo",):
    if _p not in sys.path:
        sys.path.insert(0, _p)

import numpy as np

TAU = 0.1
VTH = 1.5

B, C, H, W, T = 64, 128, 32, 32, 8
NCORES = 8
BS = B // NCORES                      # batches per core
SPAT = BS * C * H * W                 # spatial elems per core per step: 1,048,576
P = 128                               # partitions
FD = 2048                             # free dim per tile
NCH = SPAT // (P * FD)                # spatial chunks per step: 4
ROWS = T * NCH * P                    # dram rows (t-major): 4096
ELEMS = SPAT * T

_compiled = None

# v-domain scaling: v_t = 10^t * u_t kills the TAU multiply (host pre-scales
# x_t by 10^t); thresholds 1.5*10^t are all exact in f32.
VSCALE = [float(10.0**t) for t in range(T)]
VTH_T = [float(1.5 * 10.0**t) for t in range(T)]


def _build_pe(fd: int = 1024, kadd: int = 6, meng: str = "pool",
              xbufs: int = 16, mbufs: int = 8, pbufs: int = 4):
    """v-domain LIF with PE-packed output bytes.

    Recurrence per chunk c (sequential in t):
        v_t = v_{t-1} * m_{t-1} + xs_t      xs_t = 10^t * x_t (host-scaled)
        m_t = (v_t <= 1.5*10^t)             keep-mask, bf16 {0,1}  (DVE ts)
    Packing on the otherwise-idle PE: psum += (2^t I) @ m_t over the 8 steps
    gives byte = sum_t m_t 2^t (exact: bf16 holds {0,1} and 2^t; PSUM is
    f32).  Act copies PSUM -> SBUF u8; host inverts bits (o = NOT m).
    The reset multiply runs on Pool (pure-ish tt), adds split DVE/Pool via
    `kadd` (# adds per chunk on DVE).
    """
    import concourse.bacc as bacc
    import concourse.mybir as mybir
    import concourse.tile as tile

    nch = SPAT // (P * fd)
    nc = bacc.Bacc(
        "TRN2", target_bir_lowering=False, debug=False, num_devices=NCORES
    )
    f32 = mybir.dt.float32
    bf16 = mybir.dt.bfloat16
    u8 = mybir.dt.uint8
    mult = mybir.AluOpType.mult
    add = mybir.AluOpType.add
    is_le = mybir.AluOpType.is_le
    copy_f = mybir.ActivationFunctionType.Copy

    x_d = nc.dram_tensor(
        "x", [nch * T * P, fd], f32, kind="ExternalInput"
    ).ap()
    w_d = nc.dram_tensor("w", [T * P, P], bf16, kind="ExternalInput").ap()
    o_d = nc.dram_tensor("o", [nch * P, fd], u8, kind="ExternalOutput").ap()

    with tile.TileContext(nc) as tc:
        with (
            tc.tile_pool(name="xp", bufs=xbufs) as xp,
            tc.tile_pool(name="mp", bufs=mbufs) as mp,
            tc.tile_pool(name="op", bufs=2) as op_,
            tc.tile_pool(name="wp", bufs=1) as wp,
            tc.psum_pool(name="pp", bufs=pbufs) as pp,
        ):
            wts = []
            for t in range(T):
                wt = wp.tile([P, P], bf16, tag=f"w{t}", name=f"w{t}")
                nc.sync.dma_start(out=wt[:], in_=w_d[t * P : (t + 1) * P, :])
                wts.append(wt)
            for c in range(nch):
                ps = pp.tile([P, fd], f32, name="ps")
                st = None
                mprev = None
                for t in range(T):
                    r0 = (c * T + t) * P
                    xt = xp.tile([P, fd], f32)
                    nc.sync.dma_start(out=xt[:], in_=x_d[r0 : r0 + P, :])
                    if t > 0:
                        # um = v_{t-1} * m_{t-1}  (in place on state tile)
                        me = nc.gpsimd if meng == "pool" else nc.vector
                        me.tensor_tensor(
                            out=st[:], in0=st[:], in1=mprev[:], op=mult
                        )
                        # v_t = um + xs_t  (in place on the x tile)
                        ae = nc.vector if t <= kadd else nc.gpsimd
                        ae.tensor_tensor(
                            out=xt[:], in0=st[:], in1=xt[:], op=add
                        )
                    st = xt
                    m = mp.tile([P, fd], bf16)
                    nc.vector.tensor_scalar(
                        m[:], st[:], VTH_T[t], None, is_le
                    )
                    # PSUM bank limit: <=512 f32 out columns per matmul
                    for h in range(fd // 512):
                        sl = slice(h * 512, (h + 1) * 512)
                        nc.tensor.matmul(
                            ps[:, sl], wts[t][:], m[:, sl],
                            start=(t == 0), stop=(t == T - 1),
                        )
                    mprev = m
                ot = op_.tile([P, fd], u8)
                nc.scalar.activation(ot[:], ps[:], copy_f)
                nc.sync.dma_start(
                    out=o_d[c * P : (c + 1) * P, :], in_=ot[:]
                )
    nc.compile()
    return nc


def _h_ismul(t: int, c: int, nch: int, jm7: int) -> bool:
    """Static per-quantum choice: True = reset via Pool multiply (Act emits
    the keep-mask), False = reset via DVE copy_predicated (Act emits the
    spike-mask).  t is the step whose mask this is (0..T-2)."""
    return (t * nch + c) % 7 < jm7


def _build_h(fd: int = FD, jm7: int = 4, xbufs: int = 12, obufs: int = 8):
    """i8-out hybrid: per step the Act engine emits one u8 {0,1} mask tile
    (spike- or keep-oriented), which is both the reset selector and the DMA'd
    output byte (host re-inverts keep-oriented blocks).

    Per step t>0, chunk c (28 update quanta):
      cp path:   u' = 0 where o_prev      DVE copy_predicated (2.4us/q)
      mul path:  u' = u * m_prev          Pool mixed u8*f32 tt (6.0us/q)
      then       u_t = TAU*u' + x_t       DVE stt (2.3us/q)
      mask       Act Sign(+-(u-VTH))      2.0us/q
    jm7/7 of quanta take the mul path, balancing DVE ~93us / Pool ~96us
    under the 42MB DMA wall (~118us @ 356GB/s measured).
    """
    import concourse.bacc as bacc
    import concourse.mybir as mybir
    import concourse.tile as tile

    nch = SPAT // (P * fd)
    nc = bacc.Bacc(
        "TRN2", target_bir_lowering=False, debug=False, num_devices=NCORES
    )
    f32 = mybir.dt.float32
    u8 = mybir.dt.uint8
    mult = mybir.AluOpType.mult
    add = mybir.AluOpType.add
    sign_f = mybir.ActivationFunctionType.Sign

    x_d = nc.dram_tensor("x", [T * nch * P, fd], f32, kind="ExternalInput").ap()
    o_d = nc.dram_tensor("o", [T * nch * P, fd], u8, kind="ExternalOutput").ap()

    with tile.TileContext(nc) as tc:
        with (
            tc.tile_pool(name="xp", bufs=xbufs) as xp,
            tc.tile_pool(name="op", bufs=obufs) as op_,
            tc.tile_pool(name="cp", bufs=1) as cp,
        ):
            zero = cp.tile([P, fd], f32, tag="zero")
            nc.gpsimd.memset(zero[:], 0.0)
            nvth = cp.tile([P, 1], f32, tag="nvth")
            nc.gpsimd.memset(nvth[:], -VTH)
            pvth = cp.tile([P, 1], f32, tag="pvth")
            nc.gpsimd.memset(pvth[:], VTH)

            st = [None] * nch    # state tile per chunk
            mk = [None] * nch    # mask tile (u8) per chunk
            for t in range(T):
                for c in range(nch):
                    r0 = (t * nch + c) * P
                    xt = xp.tile([P, fd], f32)
                    nc.sync.dma_start(out=xt[:], in_=x_d[r0 : r0 + P, :])
                    if t > 0:
                        if _h_ismul(t - 1, c, nch, jm7):
                            # keep-mask: u' = u * m  (mixed u8*f32, Pool)
                            nc.gpsimd.tensor_tensor(
                                out=st[c][:], in0=st[c][:], in1=mk[c][:],
                                op=mult,
                            )
                        else:
                            # spike-mask: zero u where spiked (DVE)
                            nc.vector.copy_predicated(
                                out=st[c][:], mask=mk[c][:], data=zero[:]
                            )
                        # u_t = TAU*u' + x_t  (in place on x tile, DVE)
                        nc.vector.scalar_tensor_tensor(
                            out=xt[:], in0=st[c][:], scalar=TAU, in1=xt[:],
                            op0=mult, op1=add,
                        )
                    st[c] = xt
                    o = op_.tile([P, fd], u8)
                    if t < T - 1 and _h_ismul(t, c, nch, jm7):
                        # m = sign(VTH - u) -> u8 {0,1}: keep-mask
                        nc.scalar.activation(
                            o[:], st[c][:], sign_f, bias=pvth[:], scale=-1.0
                        )
                    else:
                        # o = sign(u - VTH) -> u8 {0,1}: spike-mask
                        nc.scalar.activation(
                            o[:], st[c][:], sign_f, bias=nvth[:]
                        )
                    mk[c] = o
                    nc.sync.dma_start(out=o_d[r0 : r0 + P, :], in_=o[:])
    nc.compile()
    return nc


def _build_pk(spike: str = "act", xbufs: int = 12, obufs: int = 6, fd: int = FD):
    """Bit-packed output variant: one u8 byte per spatial element holding all
    T=8 spikes (bit t = o_t), cutting output HBM traffic 32x vs f32.

    Per time step t, per [P, fd] chunk c (engine assignment in parens):
      decay   u_t = TAU*u'_{t-1} + x_t        stt, in place on x tile  (Pool)
      spike   o_t = (u_t > VTH) as u8 {0,1}   (Act: Sign(u-VTH) -> u8, the
                                               -1 saturating to 0; or DVE/Pool
                                               tensor_scalar is_gt)
      pack    acc += o_t << t                 stt, acc is the u8 out tile (DVE)
      reset   u'_t = 0 where o_t              copy_predicated, mask=o_t (DVE)
    """
    import concourse.bacc as bacc
    import concourse.mybir as mybir
    import concourse.tile as tile

    nch = SPAT // (P * fd)
    nc = bacc.Bacc(
        "TRN2", target_bir_lowering=False, debug=False, num_devices=NCORES
    )
    f32 = mybir.dt.float32
    u8 = mybir.dt.uint8
    mult = mybir.AluOpType.mult
    add = mybir.AluOpType.add
    is_gt = mybir.AluOpType.is_gt
    is_le = mybir.AluOpType.is_le
    sign_f = mybir.ActivationFunctionType.Sign

    x_d = nc.dram_tensor("x", [T * nch * P, fd], f32, kind="ExternalInput").ap()
    o_d = nc.dram_tensor("o", [nch * P, fd], u8, kind="ExternalOutput").ap()

    with tile.TileContext(nc) as tc:
        with (
            tc.tile_pool(name="xp", bufs=xbufs) as xp,
            tc.tile_pool(name="op", bufs=obufs) as op_,
            tc.tile_pool(name="cp", bufs=1) as cp,
        ):
            zero = None
            nvth = None
            if spike != "ts":
                zero = cp.tile([P, fd], f32, tag="zero")
                nc.gpsimd.memset(zero[:], 0.0)
                nvth = cp.tile([P, 1], f32, tag="nvth")
                nc.gpsimd.memset(nvth[:], -VTH)
            # f32 accumulator (Pool can't do u8+u8 adds); u8 out tile is
            # written once by the final t=7 pack op.
            acc = [
                cp.tile([P, fd], f32, tag=f"acc{c}", name=f"acc{c}")
                for c in range(nch)
            ]
            out8 = [
                cp.tile([P, fd], u8, tag=f"out{c}", name=f"out{c}")
                for c in range(nch)
            ]
            st = [None] * nch
            for t in range(T):
                for c in range(nch):
                    r0 = (t * nch + c) * P
                    xt = xp.tile([P, fd], f32)
                    nc.sync.dma_start(out=xt[:], in_=x_d[r0 : r0 + P, :])
                    if t > 0:
                        # u_t = TAU*u' + x_t  (in place on the x tile).
                        # stt is DVE-only on v3 (Pool rejects TensorScalarPtr
                        # in the stt form).
                        nc.vector.scalar_tensor_tensor(
                            out=xt[:], in0=st[c][:], scalar=TAU, in1=xt[:],
                            op0=mult, op1=add,
                        )
                    st[c] = xt
                    if spike == "ts":
                        # mask-free: weighted spike + gated state, all-DVE
                        ws = op_.tile([P, fd], f32, name="wsf")
                        nc.vector.tensor_scalar(
                            ws[:], st[c][:], VTH, float(1 << t), is_gt, mult
                        )
                        if t == 0:
                            nc.vector.tensor_scalar(
                                acc[c][:], ws[:], 1.0, None, mult
                            )
                        else:
                            dst = out8[c] if t == T - 1 else acc[c]
                            nc.vector.tensor_tensor(
                                out=dst[:], in0=ws[:], in1=acc[c][:], op=add
                            )
                        if t < T - 1:
                            # u'' = (u <= VTH) * u   (kills spiked state)
                            nc.vector.scalar_tensor_tensor(
                                out=st[c][:], in0=st[c][:], scalar=VTH,
                                in1=st[c][:], op0=is_le, op1=mult,
                            )
                    else:
                        o = op_.tile([P, fd], u8)
                        if spike == "act":
                            # o = sign(u - VTH) -> u8: -1 saturates to 0
                            nc.scalar.activation(
                                o[:], st[c][:], sign_f, bias=nvth[:]
                            )
                        else:
                            eng = nc.vector if (t + c) % 2 else nc.gpsimd
                            eng.tensor_scalar(o[:], st[c][:], VTH, None, is_gt)
                        # pack: acc (f32) += o << t.  DVE already carries
                        # decay+reset (57us floor), so t<=5 pack goes to the
                        # otherwise-idle Pool engine (u8 ts, then the legal
                        # mixed u8+f32 tt add); t=6,7 are single DVE stt ops,
                        # t=7 writing the final u8 byte.
                        if t == 0:
                            nc.gpsimd.tensor_scalar(
                                acc[c][:], o[:], 1.0, None, mult
                            )
                        elif t <= 5:
                            ws = op_.tile([P, fd], u8, name="ws")
                            nc.gpsimd.tensor_scalar(
                                ws[:], o[:], float(1 << t), None, mult
                            )
                            nc.gpsimd.tensor_tensor(
                                out=acc[c][:], in0=ws[:], in1=acc[c][:],
                                op=add,
                            )
                        else:
                            dst = out8[c] if t == T - 1 else acc[c]
                            nc.vector.scalar_tensor_tensor(
                                out=dst[:], in0=o[:], scalar=float(1 << t),
                                in1=acc[c][:], op0=mult, op1=add,
                            )
                        if t < T - 1:
                            nc.vector.copy_predicated(
                                out=st[c][:], mask=o[:], data=zero[:]
                            )
                    if t == T - 1:
                        nc.sync.dma_start(
                            out=o_d[c * P : (c + 1) * P, :], in_=out8[c][:]
                        )
    nc.compile()
    return nc


def _build(
    reps: int = 1,
    mode: str = "full",
    bufs=(10, 10),
    fd=FD,
    odt: str = "f32",
    ger: str = "v",
    mer: str = "v",
    spike: str = "dve",
    pack: bool = False,
    ib: int = 2,
):
    import contextlib

    import concourse.bacc as bacc
    import concourse.mybir as mybir
    import concourse.tile as tile

    nch = SPAT // (P * fd)
    nc = bacc.Bacc(
        "TRN2",
        target_bir_lowering=False,
        debug=False,
        num_devices=NCORES,
    )
    f32 = mybir.dt.float32
    odtype = f32 if odt == "f32" else mybir.dt.int8
    if pack:
        # in rows (t, cg, p) cols (half, j); out rows (t, p) cols (c, j)
        x_d = nc.dram_tensor(
            "x", [T * (nch // ib) * P, ib * fd], f32, kind="ExternalInput"
        ).ap()
        o_d = nc.dram_tensor(
            "o", [T * P, nch * fd], mybir.dt.int8, kind="ExternalOutput"
        ).ap()
    else:
        x_d = nc.dram_tensor(
            "x", [T * nch * P, fd], f32, kind="ExternalInput"
        ).ap()
        o_d = nc.dram_tensor(
            "o", [T * nch * P, fd], odtype, kind="ExternalOutput"
        ).ap()

    with tile.TileContext(nc) as tc:
        with (
            tc.tile_pool(name="xp", bufs=bufs[0]) as xp,
            tc.tile_pool(name="op", bufs=bufs[1]) as op_,
            tc.tile_pool(name="wp", bufs=6) as wp,
            tc.tile_pool(name="cp", bufs=1) as cp,
        ):
            rep_ctx = (
                tc.For_i(0, reps, 1) if reps > 1 else contextlib.nullcontext()
            )
            with rep_ctx:
                if pack:
                    _emit_packed(nc, xp, op_, cp, x_d, o_d, mybir, mode,
                                 fd, nch, ib)
                else:
                    _emit(nc, xp, op_, wp, cp, x_d, o_d, mybir, mode, fd,
                          nch, odt, ger, mer, spike)
    nc.compile()
    return nc


def _emit_packed(nc, xp, op_, cp, x_d, o_d, mybir, mode, fd, nch, ib):
    """act1-spike i8-out variant with batched DMAs.

    Input tiles span `ib` chunks (one contiguous DMA each); output tiles
    span all `nch` chunks of a step (one contiguous DMA per step).
    """
    f32 = mybir.dt.float32
    i8 = mybir.dt.int8
    mult = mybir.AluOpType.mult
    add = mybir.AluOpType.add
    relu_f = mybir.ActivationFunctionType.Relu
    dma, compute = mode in ("full", "dma"), mode in ("full", "compute")
    ng = nch // ib

    zero = cp.tile([P, fd], f32, tag="zero")
    nc.gpsimd.memset(zero[:], 0.0)
    nvthbig = cp.tile([P, 1], f32, tag="nvthbig")
    nc.gpsimd.memset(nvthbig[:], -VTH * 1e9)

    u = [None] * nch       # AP slice holding u_t per chunk
    o_prev = [None] * nch  # AP slice of o_{t-1} per chunk
    for t in range(T):
        xts = []
        for g in range(ng):
            xt = xp.tile([P, ib * fd], f32)
            if dma:
                r0 = (t * ng + g) * P
                nc.sync.dma_start(out=xt[:], in_=x_d[r0 : r0 + P, :])
            elif t == 0:
                nc.gpsimd.memset(xt[:], 0.25)
            xts.append(xt)
        ot = op_.tile([P, nch * fd], i8)
        for c in range(nch):
            g, h = c // ib, c % ib
            xs = xts[g][:, h * fd : (h + 1) * fd]
            if compute:
                if t > 0:
                    # reset where previous step spiked
                    nc.vector.copy_predicated(
                        out=u[c], mask=o_prev[c], data=zero[:]
                    )
                    # u_t = TAU*u_masked + x_t  (in place on x slice)
                    nc.vector.scalar_tensor_tensor(
                        out=xs, in0=u[c], scalar=TAU, in1=xs,
                        op0=mult, op1=add,
                    )
                u[c] = xs
                # o8 = sat_i8(relu(1e9*u - 1e9*VTH)): nonzero iff spike
                nc.scalar.activation(
                    ot[:, c * fd : (c + 1) * fd], u[c], relu_f,
                    bias=nvthbig[:], scale=1e9,
                )
                o_prev[c] = ot[:, c * fd : (c + 1) * fd]
        if not compute:
            nc.gpsimd.memset(ot[:, :1], 1)
        if dma:
            nc.sync.dma_start(out=o_d[t * P : (t + 1) * P, :], in_=ot[:])


def _emit(nc, xp, op_, wp, cp, x_d, o_d, mybir, mode, fd, nch, odt, ger, mer,
          spike="dve"):
    f32 = mybir.dt.float32
    mult = mybir.AluOpType.mult
    add = mybir.AluOpType.add
    is_gt = mybir.AluOpType.is_gt
    copy_f = mybir.ActivationFunctionType.Copy
    dma, compute = mode in ("full", "dma"), mode in ("full", "compute")
    odtype = f32 if odt == "f32" else mybir.dt.int8
    geng = nc.vector if ger == "v" else nc.gpsimd
    meng = nc.vector if mer == "v" else nc.gpsimd

    i8 = mybir.dt.int8
    relu_f = mybir.ActivationFunctionType.Relu
    sign_f = mybir.ActivationFunctionType.Sign

    o8c = None
    if mode == "dma" and odt == "i8":
        o8c = cp.tile([P, fd], i8, tag="o8c")
        nc.gpsimd.memset(o8c[:], 1)
    if spike in ("act", "act1"):
        assert odt == "i8"
        zero = cp.tile([P, fd], f32, tag="zero")
        nc.gpsimd.memset(zero[:], 0.0)
        nvth = cp.tile([P, 1], f32, tag="nvth")
        nc.gpsimd.memset(nvth[:], -VTH)
        nvthbig = cp.tile([P, 1], f32, tag="nvthbig")
        nc.gpsimd.memset(nvthbig[:], -VTH * 1e9)

    u = [None] * nch       # tile holding u_t per chunk
    o_prev = [None] * nch  # tile holding o_{t-1} per chunk
    for t in range(T):
        for c in range(nch):
            r0 = (t * nch + c) * P
            xt = xp.tile([P, fd], f32)
            if dma:
                nc.sync.dma_start(out=xt[:], in_=x_d[r0 : r0 + P, :])
            elif t == 0:
                nc.gpsimd.memset(xt[:], 0.25)
            if compute and spike in ("act", "act1"):
                if t > 0:
                    o = o_prev[c]
                    # reset where previous step spiked
                    nc.vector.copy_predicated(
                        out=u[c][:], mask=o[:], data=zero[:]
                    )
                    # u_t = TAU*u_masked + x_t  (in place on x tile)
                    nc.vector.scalar_tensor_tensor(
                        out=xt[:], in0=u[c][:], scalar=TAU, in1=xt[:],
                        op0=mult, op1=add,
                    )
                u[c] = xt
                ot = op_.tile([P, fd], i8)
                if spike == "act1":
                    # o8 = sat_i8(relu(1e9*u - 1e9*VTH)): nonzero iff spike.
                    # int8 conversion saturates at 127 (verified on HW), and
                    # |u-VTH| >= 1 ulp(1.5) so the *1e9 rounding never
                    # crosses zero.
                    nc.scalar.activation(
                        ot[:], u[c][:], relu_f, bias=nvthbig[:], scale=1e9
                    )
                else:
                    # spike on ScalarE: sg = sign(u - VTH); o = relu(sg)
                    sg = wp.tile([P, fd], f32, tag="sg")
                    nc.scalar.activation(sg[:], u[c][:], sign_f, bias=nvth[:])
                    nc.scalar.activation(ot[:], sg[:], relu_f)
                o_prev[c] = ot
            elif compute:
                if t == 0:
                    u[c] = xt
                else:
                    o = o_prev[c]
                    if odt == "f32":
                        # w <- TAU - TAU*o  (in place over o after its store)
                        w = o
                        nc.scalar.activation(
                            w[:], o[:], copy_f, bias=TAU, scale=-TAU
                        )
                    else:
                        w = wp.tile([P, fd], f32)
                        nc.scalar.activation(
                            w[:], o[:], copy_f, bias=TAU, scale=-TAU
                        )
                    # u_masked = u_{t-1} * w   (in place)
                    meng.tensor_tensor(
                        out=u[c][:], in0=u[c][:], in1=w[:], op=mult
                    )
                    # u_t = u_masked + x_t    (in place on x tile)
                    nc.vector.tensor_tensor(
                        out=xt[:], in0=u[c][:], in1=xt[:], op=add
                    )
                    u[c] = xt
                ot = op_.tile([P, fd], odtype)
                geng.tensor_scalar(ot[:], u[c][:], VTH, None, is_gt)
                o_prev[c] = ot
            else:
                ot = o8c if o8c is not None else xt
            if dma:
                nc.sync.dma_start(out=o_d[r0 : r0 + P, :], in_=ot[:])


def _mode():
    import os

    return os.environ.get("LIF_MODE", "pe")


def _get_compiled():
    global _compiled
    if _compiled is None:
        import os

        mode = _mode()
        if mode == "pe":
            _compiled = _build_pe(
                fd=int(os.environ.get("LIF_FD", "1024")),
                kadd=int(os.environ.get("LIF_KADD", "6")),
                meng=os.environ.get("LIF_MENG", "pool"),
                xbufs=int(os.environ.get("LIF_XBUFS", "16")),
                pbufs=int(os.environ.get("LIF_PBUFS", "4")),
            )
        elif mode == "pk":
            _compiled = _build_pk(
                spike=os.environ.get("LIF_SPIKE", "act"),
                xbufs=int(os.environ.get("LIF_XBUFS", "12")),
                obufs=int(os.environ.get("LIF_OBUFS", "6")),
            )
        elif mode == "act1":
            _compiled = _build(spike="act1", odt="i8")
        elif mode == "h":
            _compiled = _build_h(
                jm7=int(os.environ.get("LIF_JM7", "4")),
                xbufs=int(os.environ.get("LIF_XBUFS", "12")),
                obufs=int(os.environ.get("LIF_OBUFS", "8")),
            )
        else:
            _compiled = _build()
    return _compiled


def _shard_pe(x: np.ndarray, i: int, fd: int) -> np.ndarray:
    """Core i's shard for pe mode: chunk-major [(c,t,p), fd] rows, with
    x_t pre-scaled by 10^t (v-domain)."""
    nch = SPAT // (P * fd)
    xs = x[i * BS : (i + 1) * BS].reshape(SPAT, T)
    xv = xs * np.asarray(VSCALE, dtype=np.float32)[None, :]
    xv = xv.reshape(nch, P, fd, T).transpose(0, 3, 1, 2)  # [c, t, P, fd]
    return np.ascontiguousarray(xv).reshape(nch * T * P, fd)


def _w_pe() -> np.ndarray:
    import ml_dtypes

    eye = np.eye(P, dtype=np.float32)
    w = np.concatenate([eye * float(1 << t) for t in range(T)], axis=0)
    return w.astype(ml_dtypes.bfloat16)


def _shard_tmajor(x: np.ndarray, i: int) -> np.ndarray:
    """Core i's shard, time-major: [T*NCH*P, FD], row-major over (t, spatial)."""
    xs = x[i * BS : (i + 1) * BS]                   # [BS,C,H,W,T]
    xt = np.moveaxis(xs.reshape(SPAT, T), -1, 0)    # [T, SPAT]
    return np.ascontiguousarray(xt).reshape(ROWS, FD)


def kernel(x: np.ndarray, _trace: bool = False):
    nc = _get_compiled()
    from concourse.bass_utils import run_bass_kernel_spmd

    x = np.asarray(x, dtype=np.float32)
    if _mode() == "pe":
        import os

        fd = int(os.environ.get("LIF_FD", "1024"))
        w = _w_pe()
        in_maps = [
            {"x": _shard_pe(x, i, fd), "w": w} for i in range(NCORES)
        ]
    else:
        in_maps = [{"x": _shard_tmajor(x, i)} for i in range(NCORES)]
    res = run_bass_kernel_spmd(
        nc, in_maps, core_ids=list(range(NCORES)), trace=_trace
    )
    invert = _mode() == "pe"                        # pe packs the keep-mask
    outs = []
    for r in res.results:
        ot = r["o"]
        if ot.size == SPAT:                         # bit-packed u8: bit t = o_t
            if invert:
                ot = np.invert(ot)
            bits = np.unpackbits(
                ot.reshape(-1, 1), axis=1, bitorder="little"
            )[:, :T]
            outs.append(bits.reshape(BS, C, H, W, T).astype(np.float32))
            continue
        if _mode() == "h":                          # u8 masks, mixed polarity
            import os

            jm7 = int(os.environ.get("LIF_JM7", "4"))
            nch = SPAT // (P * FD)
            bits = (ot != 0).reshape(T, nch, P, FD)
            for t in range(T - 1):
                for c in range(nch):
                    if _h_ismul(t, c, nch, jm7):
                        bits[t, c] = ~bits[t, c]
            ot = bits.reshape(T, SPAT).astype(np.float32)
            outs.append(np.moveaxis(ot, 0, -1).reshape(BS, C, H, W, T))
            continue
        if ot.dtype != np.float32:                  # int8 spikes -> f32
            ot = (ot != 0).astype(np.float32)
        ot = ot.reshape(T, SPAT)                    # time-major back to T-last
        outs.append(np.moveaxis(ot, 0, -1).reshape(BS, C, H, W, T))
    out = np.ascontiguousarray(np.concatenate(outs, axis=0))
    return (out, res) if _trace else out



# revision 24
# speedup vs baseline: 5.2658x; 1.0014x over previous
"""LIF spike (leaky integrate-and-fire) forward kernel for Trainium2.

Recurrence over the time axis T=8 of x[64,128,32,32,8] (fp32):
    u_t = TAU * u_{t-1} * (1 - o_{t-1}) + x_t
    o_t = (u_t > VTH)
Data-parallel over the batch dim: 8 NeuronCores x 8 batches each.

Layout: the host transposes each core's shard to time-major [T, spatial]
so that every time-step slice is a contiguous [128, FD] tile (unit-stride
APs for every engine op, contiguous >=1MiB DMAs). Per step the work is:
    o_t  = (u_t > VTH)                 DVE tensor_scalar is_gt -> fp32 out
    w_t  = TAU - TAU*o_t               ScalarE activation Copy(scale,bias),
                                       written in place over o_t after its
                                       store DMA has read it
    u_'  = u_t * w_t                   DVE tensor_tensor mult (in place)
    u_t1 = u_' + x_t1                  DVE tensor_tensor add (in place on
                                       the freshly loaded x tile)
The x tile doubles as the membrane-state buffer, the o tile doubles as the
w buffer, so SBUF holds just two fp32 pools.
"""

import sys

for _p in ("/opt/trn_rl_repo",):
    if _p not in sys.path:
        sys.path.insert(0, _p)

import numpy as np

TAU = 0.1
VTH = 1.5

B, C, H, W, T = 64, 128, 32, 32, 8
NCORES = 8
BS = B // NCORES                      # batches per core
SPAT = BS * C * H * W                 # spatial elems per core per step: 1,048,576
P = 128                               # partitions
FD = 2048                             # free dim per tile
NCH = SPAT // (P * FD)                # spatial chunks per step: 4
ROWS = T * NCH * P                    # dram rows (t-major): 4096
ELEMS = SPAT * T

_compiled = None

# v-domain scaling: v_t = 10^t * u_t kills the TAU multiply (host pre-scales
# x_t by 10^t); thresholds 1.5*10^t are all exact in f32.
VSCALE = [float(10.0**t) for t in range(T)]
VTH_T = [float(1.5 * 10.0**t) for t in range(T)]


def _build_pe(fd: int = 1024, kadd: int = 6, meng: str = "pool",
              xbufs: int = 16, mbufs: int = 8, pbufs: int = 4):
    """v-domain LIF with PE-packed output bytes.

    Recurrence per chunk c (sequential in t):
        v_t = v_{t-1} * m_{t-1} + xs_t      xs_t = 10^t * x_t (host-scaled)
        m_t = (v_t <= 1.5*10^t)             keep-mask, bf16 {0,1}  (DVE ts)
    Packing on the otherwise-idle PE: psum += (2^t I) @ m_t over the 8 steps
    gives byte = sum_t m_t 2^t (exact: bf16 holds {0,1} and 2^t; PSUM is
    f32).  Act copies PSUM -> SBUF u8; host inverts bits (o = NOT m).
    The reset multiply runs on Pool (pure-ish tt), adds split DVE/Pool via
    `kadd` (# adds per chunk on DVE).
    """
    import concourse.bacc as bacc
    import concourse.mybir as mybir
    import concourse.tile as tile

    nch = SPAT // (P * fd)
    nc = bacc.Bacc(
        "TRN2", target_bir_lowering=False, debug=False, num_devices=NCORES
    )
    f32 = mybir.dt.float32
    bf16 = mybir.dt.bfloat16
    u8 = mybir.dt.uint8
    mult = mybir.AluOpType.mult
    add = mybir.AluOpType.add
    is_le = mybir.AluOpType.is_le
    copy_f = mybir.ActivationFunctionType.Copy

    x_d = nc.dram_tensor(
        "x", [nch * T * P, fd], f32, kind="ExternalInput"
    ).ap()
    w_d = nc.dram_tensor("w", [T * P, P], bf16, kind="ExternalInput").ap()
    o_d = nc.dram_tensor("o", [nch * P, fd], u8, kind="ExternalOutput").ap()

    with tile.TileContext(nc) as tc:
        with (
            tc.tile_pool(name="xp", bufs=xbufs) as xp,
            tc.tile_pool(name="mp", bufs=mbufs) as mp,
            tc.tile_pool(name="op", bufs=2) as op_,
            tc.tile_pool(name="wp", bufs=1) as wp,
            tc.psum_pool(name="pp", bufs=pbufs) as pp,
        ):
            wts = []
            for t in range(T):
                wt = wp.tile([P, P], bf16, tag=f"w{t}", name=f"w{t}")
                nc.sync.dma_start(out=wt[:], in_=w_d[t * P : (t + 1) * P, :])
                wts.append(wt)
            for c in range(nch):
                ps = pp.tile([P, fd], f32, name="ps")
                st = None
                mprev = None
                for t in range(T):
                    r0 = (c * T + t) * P
                    xt = xp.tile([P, fd], f32)
                    nc.sync.dma_start(out=xt[:], in_=x_d[r0 : r0 + P, :])
                    if t > 0:
                        # um = v_{t-1} * m_{t-1}  (in place on state tile)
                        me = nc.gpsimd if meng == "pool" else nc.vector
                        me.tensor_tensor(
                            out=st[:], in0=st[:], in1=mprev[:], op=mult
                        )
                        # v_t = um + xs_t  (in place on the x tile)
                        ae = nc.vector if t <= kadd else nc.gpsimd
                        ae.tensor_tensor(
                            out=xt[:], in0=st[:], in1=xt[:], op=add
                        )
                    st = xt
                    m = mp.tile([P, fd], bf16)
                    nc.vector.tensor_scalar(
                        m[:], st[:], VTH_T[t], None, is_le
                    )
                    # PSUM bank limit: <=512 f32 out columns per matmul
                    for h in range(fd // 512):
                        sl = slice(h * 512, (h + 1) * 512)
                        nc.tensor.matmul(
                            ps[:, sl], wts[t][:], m[:, sl],
                            start=(t == 0), stop=(t == T - 1),
                        )
                    mprev = m
                ot = op_.tile([P, fd], u8)
                nc.scalar.activation(ot[:], ps[:], copy_f)
                nc.sync.dma_start(
                    out=o_d[c * P : (c + 1) * P, :], in_=ot[:]
                )
    nc.compile()
    return nc


def _h_ismul(t: int, c: int, nch: int, jm7: int) -> bool:
    """Static per-quantum choice: True = reset via Pool multiply (Act emits
    the keep-mask), False = reset via DVE copy_predicated (Act emits the
    spike-mask).  t is the step whose mask this is (0..T-2)."""
    return (t * nch + c) % 7 < jm7


def _build_h(fd: int = FD, jm7: int = 4, xbufs: int = 12, obufs: int = 8):
    """i8-out hybrid: per step the Act engine emits one u8 {0,1} mask tile
    (spike- or keep-oriented), which is both the reset selector and the DMA'd
    output byte (host re-inverts keep-oriented blocks).

    Per step t>0, chunk c (28 update quanta):
      cp path:   u' = 0 where o_prev      DVE copy_predicated (2.4us/q)
      mul path:  u' = u * m_prev          Pool mixed u8*f32 tt (6.0us/q)
      then       u_t = TAU*u' + x_t       DVE stt (2.3us/q)
      mask       Act Sign(+-(u-VTH))      2.0us/q
    jm7/7 of quanta take the mul path, balancing DVE ~93us / Pool ~96us
    under the 42MB DMA wall (~118us @ 356GB/s measured).
    """
    import concourse.bacc as bacc
    import concourse.mybir as mybir
    import concourse.tile as tile

    nch = SPAT // (P * fd)
    nc = bacc.Bacc(
        "TRN2", target_bir_lowering=False, debug=False, num_devices=NCORES
    )
    f32 = mybir.dt.float32
    u8 = mybir.dt.uint8
    mult = mybir.AluOpType.mult
    add = mybir.AluOpType.add
    sign_f = mybir.ActivationFunctionType.Sign

    x_d = nc.dram_tensor("x", [T * nch * P, fd], f32, kind="ExternalInput").ap()
    # one fat row block per step: 8KB rows for efficient output DMA
    o_d = nc.dram_tensor("o", [T * P, nch * fd], u8, kind="ExternalOutput").ap()

    with tile.TileContext(nc) as tc:
        with (
            tc.tile_pool(name="xp", bufs=xbufs) as xp,
            tc.tile_pool(name="op", bufs=obufs) as op_,
            tc.tile_pool(name="cp", bufs=1) as cp,
        ):
            zero = cp.tile([P, fd], f32, tag="zero")
            nc.gpsimd.memset(zero[:], 0.0)
            nvth = cp.tile([P, 1], f32, tag="nvth")
            nc.gpsimd.memset(nvth[:], -VTH)
            pvth = cp.tile([P, 1], f32, tag="pvth")
            nc.gpsimd.memset(pvth[:], VTH)

            st = [None] * nch    # state tile per chunk
            mk = [None] * nch    # mask AP (slice of the staging tile)
            for t in range(T):
                ot = op_.tile([P, nch * fd], u8)
                for c in range(nch):
                    r0 = (t * nch + c) * P
                    xt = xp.tile([P, fd], f32)
                    nc.sync.dma_start(out=xt[:], in_=x_d[r0 : r0 + P, :])
                    if t > 0:
                        if _h_ismul(t - 1, c, nch, jm7):
                            # keep-mask: u' = u * m  (mixed u8*f32, Pool)
                            nc.gpsimd.tensor_tensor(
                                out=st[c][:], in0=st[c][:], in1=mk[c],
                                op=mult,
                            )
                        else:
                            # spike-mask: zero u where spiked (DVE)
                            nc.vector.copy_predicated(
                                out=st[c][:], mask=mk[c], data=zero[:]
                            )
                        # u_t = TAU*u' + x_t  (in place on x tile, DVE)
                        nc.vector.scalar_tensor_tensor(
                            out=xt[:], in0=st[c][:], scalar=TAU, in1=xt[:],
                            op0=mult, op1=add,
                        )
                    st[c] = xt
                    o = ot[:, c * fd : (c + 1) * fd]
                    if t < T - 1 and _h_ismul(t, c, nch, jm7):
                        # m = sign(VTH - u) -> u8 {0,1}: keep-mask
                        nc.scalar.activation(
                            o, st[c][:], sign_f, bias=pvth[:], scale=-1.0
                        )
                    else:
                        # o = sign(u - VTH) -> u8 {0,1}: spike-mask
                        nc.scalar.activation(
                            o, st[c][:], sign_f, bias=nvth[:]
                        )
                    mk[c] = o
                nc.sync.dma_start(
                    out=o_d[t * P : (t + 1) * P, :], in_=ot[:]
                )
    nc.compile()
    return nc


def _build_pk(spike: str = "act", xbufs: int = 12, obufs: int = 6, fd: int = FD):
    """Bit-packed output variant: one u8 byte per spatial element holding all
    T=8 spikes (bit t = o_t), cutting output HBM traffic 32x vs f32.

    Per time step t, per [P, fd] chunk c (engine assignment in parens):
      decay   u_t = TAU*u'_{t-1} + x_t        stt, in place on x tile  (Pool)
      spike   o_t = (u_t > VTH) as u8 {0,1}   (Act: Sign(u-VTH) -> u8, the
                                               -1 saturating to 0; or DVE/Pool
                                               tensor_scalar is_gt)
      pack    acc += o_t << t                 stt, acc is the u8 out tile (DVE)
      reset   u'_t = 0 where o_t              copy_predicated, mask=o_t (DVE)
    """
    import concourse.bacc as bacc
    import concourse.mybir as mybir
    import concourse.tile as tile

    nch = SPAT // (P * fd)
    nc = bacc.Bacc(
        "TRN2", target_bir_lowering=False, debug=False, num_devices=NCORES
    )
    f32 = mybir.dt.float32
    u8 = mybir.dt.uint8
    mult = mybir.AluOpType.mult
    add = mybir.AluOpType.add
    is_gt = mybir.AluOpType.is_gt
    is_le = mybir.AluOpType.is_le
    sign_f = mybir.ActivationFunctionType.Sign

    x_d = nc.dram_tensor("x", [T * nch * P, fd], f32, kind="ExternalInput").ap()
    o_d = nc.dram_tensor("o", [nch * P, fd], u8, kind="ExternalOutput").ap()

    with tile.TileContext(nc) as tc:
        with (
            tc.tile_pool(name="xp", bufs=xbufs) as xp,
            tc.tile_pool(name="op", bufs=obufs) as op_,
            tc.tile_pool(name="cp", bufs=1) as cp,
        ):
            zero = None
            nvth = None
            if spike != "ts":
                zero = cp.tile([P, fd], f32, tag="zero")
                nc.gpsimd.memset(zero[:], 0.0)
                nvth = cp.tile([P, 1], f32, tag="nvth")
                nc.gpsimd.memset(nvth[:], -VTH)
            # f32 accumulator (Pool can't do u8+u8 adds); u8 out tile is
            # written once by the final t=7 pack op.
            acc = [
                cp.tile([P, fd], f32, tag=f"acc{c}", name=f"acc{c}")
                for c in range(nch)
            ]
            out8 = [
                cp.tile([P, fd], u8, tag=f"out{c}", name=f"out{c}")
                for c in range(nch)
            ]
            st = [None] * nch
            for t in range(T):
                for c in range(nch):
                    r0 = (t * nch + c) * P
                    xt = xp.tile([P, fd], f32)
                    nc.sync.dma_start(out=xt[:], in_=x_d[r0 : r0 + P, :])
                    if t > 0:
                        # u_t = TAU*u' + x_t  (in place on the x tile).
                        # stt is DVE-only on v3 (Pool rejects TensorScalarPtr
                        # in the stt form).
                        nc.vector.scalar_tensor_tensor(
                            out=xt[:], in0=st[c][:], scalar=TAU, in1=xt[:],
                            op0=mult, op1=add,
                        )
                    st[c] = xt
                    if spike == "ts":
                        # mask-free: weighted spike + gated state, all-DVE
                        ws = op_.tile([P, fd], f32, name="wsf")
                        nc.vector.tensor_scalar(
                            ws[:], st[c][:], VTH, float(1 << t), is_gt, mult
                        )
                        if t == 0:
                            nc.vector.tensor_scalar(
                                acc[c][:], ws[:], 1.0, None, mult
                            )
                        else:
                            dst = out8[c] if t == T - 1 else acc[c]
                            nc.vector.tensor_tensor(
                                out=dst[:], in0=ws[:], in1=acc[c][:], op=add
                            )
                        if t < T - 1:
                            # u'' = (u <= VTH) * u   (kills spiked state)
                            nc.vector.scalar_tensor_tensor(
                                out=st[c][:], in0=st[c][:], scalar=VTH,
                                in1=st[c][:], op0=is_le, op1=mult,
                            )
                    else:
                        o = op_.tile([P, fd], u8)
                        if spike == "act":
                            # o = sign(u - VTH) -> u8: -1 saturates to 0
                            nc.scalar.activation(
                                o[:], st[c][:], sign_f, bias=nvth[:]
                            )
                        else:
                            eng = nc.vector if (t + c) % 2 else nc.gpsimd
                            eng.tensor_scalar(o[:], st[c][:], VTH, None, is_gt)
                        # pack: acc (f32) += o << t.  DVE already carries
                        # decay+reset (57us floor), so t<=5 pack goes to the
                        # otherwise-idle Pool engine (u8 ts, then the legal
                        # mixed u8+f32 tt add); t=6,7 are single DVE stt ops,
                        # t=7 writing the final u8 byte.
                        if t == 0:
                            nc.gpsimd.tensor_scalar(
                                acc[c][:], o[:], 1.0, None, mult
                            )
                        elif t <= 5:
                            ws = op_.tile([P, fd], u8, name="ws")
                            nc.gpsimd.tensor_scalar(
                                ws[:], o[:], float(1 << t), None, mult
                            )
                            nc.gpsimd.tensor_tensor(
                                out=acc[c][:], in0=ws[:], in1=acc[c][:],
                                op=add,
                            )
                        else:
                            dst = out8[c] if t == T - 1 else acc[c]
                            nc.vector.scalar_tensor_tensor(
                                out=dst[:], in0=o[:], scalar=float(1 << t),
                                in1=acc[c][:], op0=mult, op1=add,
                            )
                        if t < T - 1:
                            nc.vector.copy_predicated(
                                out=st[c][:], mask=o[:], data=zero[:]
                            )
                    if t == T - 1:
                        nc.sync.dma_start(
                            out=o_d[c * P : (c + 1) * P, :], in_=out8[c][:]
                        )
    nc.compile()
    return nc


def _build(
    reps: int = 1,
    mode: str = "full",
    bufs=(10, 10),
    fd=FD,
    odt: str = "f32",
    ger: str = "v",
    mer: str = "v",
    spike: str = "dve",
    pack: bool = False,
    ib: int = 2,
):
    import contextlib

    import concourse.bacc as bacc
    import concourse.mybir as mybir
    import concourse.tile as tile

    nch = SPAT // (P * fd)
    nc = bacc.Bacc(
        "TRN2",
        target_bir_lowering=False,
        debug=False,
        num_devices=NCORES,
    )
    f32 = mybir.dt.float32
    odtype = f32 if odt == "f32" else mybir.dt.int8
    if pack:
        # in rows (t, cg, p) cols (half, j); out rows (t, p) cols (c, j)
        x_d = nc.dram_tensor(
            "x", [T * (nch // ib) * P, ib * fd], f32, kind="ExternalInput"
        ).ap()
        o_d = nc.dram_tensor(
            "o", [T * P, nch * fd], mybir.dt.int8, kind="ExternalOutput"
        ).ap()
    else:
        x_d = nc.dram_tensor(
            "x", [T * nch * P, fd], f32, kind="ExternalInput"
        ).ap()
        o_d = nc.dram_tensor(
            "o", [T * nch * P, fd], odtype, kind="ExternalOutput"
        ).ap()

    with tile.TileContext(nc) as tc:
        with (
            tc.tile_pool(name="xp", bufs=bufs[0]) as xp,
            tc.tile_pool(name="op", bufs=bufs[1]) as op_,
            tc.tile_pool(name="wp", bufs=6) as wp,
            tc.tile_pool(name="cp", bufs=1) as cp,
        ):
            rep_ctx = (
                tc.For_i(0, reps, 1) if reps > 1 else contextlib.nullcontext()
            )
            with rep_ctx:
                if pack:
                    _emit_packed(nc, xp, op_, cp, x_d, o_d, mybir, mode,
                                 fd, nch, ib)
                else:
                    _emit(nc, xp, op_, wp, cp, x_d, o_d, mybir, mode, fd,
                          nch, odt, ger, mer, spike)
    nc.compile()
    return nc


def _emit_packed(nc, xp, op_, cp, x_d, o_d, mybir, mode, fd, nch, ib):
    """act1-spike i8-out variant with batched DMAs.

    Input tiles span `ib` chunks (one contiguous DMA each); output tiles
    span all `nch` chunks of a step (one contiguous DMA per step).
    """
    f32 = mybir.dt.float32
    i8 = mybir.dt.int8
    mult = mybir.AluOpType.mult
    add = mybir.AluOpType.add
    relu_f = mybir.ActivationFunctionType.Relu
    dma, compute = mode in ("full", "dma"), mode in ("full", "compute")
    ng = nch // ib

    zero = cp.tile([P, fd], f32, tag="zero")
    nc.gpsimd.memset(zero[:], 0.0)
    nvthbig = cp.tile([P, 1], f32, tag="nvthbig")
    nc.gpsimd.memset(nvthbig[:], -VTH * 1e9)

    u = [None] * nch       # AP slice holding u_t per chunk
    o_prev = [None] * nch  # AP slice of o_{t-1} per chunk
    for t in range(T):
        xts = []
        for g in range(ng):
            xt = xp.tile([P, ib * fd], f32)
            if dma:
                r0 = (t * ng + g) * P
                nc.sync.dma_start(out=xt[:], in_=x_d[r0 : r0 + P, :])
            elif t == 0:
                nc.gpsimd.memset(xt[:], 0.25)
            xts.append(xt)
        ot = op_.tile([P, nch * fd], i8)
        for c in range(nch):
            g, h = c // ib, c % ib
            xs = xts[g][:, h * fd : (h + 1) * fd]
            if compute:
                if t > 0:
                    # reset where previous step spiked
                    nc.vector.copy_predicated(
                        out=u[c], mask=o_prev[c], data=zero[:]
                    )
                    # u_t = TAU*u_masked + x_t  (in place on x slice)
                    nc.vector.scalar_tensor_tensor(
                        out=xs, in0=u[c], scalar=TAU, in1=xs,
                        op0=mult, op1=add,
                    )
                u[c] = xs
                # o8 = sat_i8(relu(1e9*u - 1e9*VTH)): nonzero iff spike
                nc.scalar.activation(
                    ot[:, c * fd : (c + 1) * fd], u[c], relu_f,
                    bias=nvthbig[:], scale=1e9,
                )
                o_prev[c] = ot[:, c * fd : (c + 1) * fd]
        if not compute:
            nc.gpsimd.memset(ot[:, :1], 1)
        if dma:
            nc.sync.dma_start(out=o_d[t * P : (t + 1) * P, :], in_=ot[:])


def _emit(nc, xp, op_, wp, cp, x_d, o_d, mybir, mode, fd, nch, odt, ger, mer,
          spike="dve"):
    f32 = mybir.dt.float32
    mult = mybir.AluOpType.mult
    add = mybir.AluOpType.add
    is_gt = mybir.AluOpType.is_gt
    copy_f = mybir.ActivationFunctionType.Copy
    dma, compute = mode in ("full", "dma"), mode in ("full", "compute")
    odtype = f32 if odt == "f32" else mybir.dt.int8
    geng = nc.vector if ger == "v" else nc.gpsimd
    meng = nc.vector if mer == "v" else nc.gpsimd

    i8 = mybir.dt.int8
    relu_f = mybir.ActivationFunctionType.Relu
    sign_f = mybir.ActivationFunctionType.Sign

    o8c = None
    if mode == "dma" and odt == "i8":
        o8c = cp.tile([P, fd], i8, tag="o8c")
        nc.gpsimd.memset(o8c[:], 1)
    if spike in ("act", "act1"):
        assert odt == "i8"
        zero = cp.tile([P, fd], f32, tag="zero")
        nc.gpsimd.memset(zero[:], 0.0)
        nvth = cp.tile([P, 1], f32, tag="nvth")
        nc.gpsimd.memset(nvth[:], -VTH)
        nvthbig = cp.tile([P, 1], f32, tag="nvthbig")
        nc.gpsimd.memset(nvthbig[:], -VTH * 1e9)

    u = [None] * nch       # tile holding u_t per chunk
    o_prev = [None] * nch  # tile holding o_{t-1} per chunk
    for t in range(T):
        for c in range(nch):
            r0 = (t * nch + c) * P
            xt = xp.tile([P, fd], f32)
            if dma:
                nc.sync.dma_start(out=xt[:], in_=x_d[r0 : r0 + P, :])
            elif t == 0:
                nc.gpsimd.memset(xt[:], 0.25)
            if compute and spike in ("act", "act1"):
                if t > 0:
                    o = o_prev[c]
                    # reset where previous step spiked
                    nc.vector.copy_predicated(
                        out=u[c][:], mask=o[:], data=zero[:]
                    )
                    # u_t = TAU*u_masked + x_t  (in place on x tile)
                    nc.vector.scalar_tensor_tensor(
                        out=xt[:], in0=u[c][:], scalar=TAU, in1=xt[:],
                        op0=mult, op1=add,
                    )
                u[c] = xt
                ot = op_.tile([P, fd], i8)
                if spike == "act1":
                    # o8 = sat_i8(relu(1e9*u - 1e9*VTH)): nonzero iff spike.
                    # int8 conversion saturates at 127 (verified on HW), and
                    # |u-VTH| >= 1 ulp(1.5) so the *1e9 rounding never
                    # crosses zero.
                    nc.scalar.activation(
                        ot[:], u[c][:], relu_f, bias=nvthbig[:], scale=1e9
                    )
                else:
                    # spike on ScalarE: sg = sign(u - VTH); o = relu(sg)
                    sg = wp.tile([P, fd], f32, tag="sg")
                    nc.scalar.activation(sg[:], u[c][:], sign_f, bias=nvth[:])
                    nc.scalar.activation(ot[:], sg[:], relu_f)
                o_prev[c] = ot
            elif compute:
                if t == 0:
                    u[c] = xt
                else:
                    o = o_prev[c]
                    if odt == "f32":
                        # w <- TAU - TAU*o  (in place over o after its store)
                        w = o
                        nc.scalar.activation(
                            w[:], o[:], copy_f, bias=TAU, scale=-TAU
                        )
                    else:
                        w = wp.tile([P, fd], f32)
                        nc.scalar.activation(
                            w[:], o[:], copy_f, bias=TAU, scale=-TAU
                        )
                    # u_masked = u_{t-1} * w   (in place)
                    meng.tensor_tensor(
                        out=u[c][:], in0=u[c][:], in1=w[:], op=mult
                    )
                    # u_t = u_masked + x_t    (in place on x tile)
                    nc.vector.tensor_tensor(
                        out=xt[:], in0=u[c][:], in1=xt[:], op=add
                    )
                    u[c] = xt
                ot = op_.tile([P, fd], odtype)
                geng.tensor_scalar(ot[:], u[c][:], VTH, None, is_gt)
                o_prev[c] = ot
            else:
                ot = o8c if o8c is not None else xt
            if dma:
                nc.sync.dma_start(out=o_d[r0 : r0 + P, :], in_=ot[:])


def _mode():
    import os

    return os.environ.get("LIF_MODE", "pe")


def _get_compiled():
    global _compiled
    if _compiled is None:
        import os

        mode = _mode()
        if mode == "pe":
            _compiled = _build_pe(
                fd=int(os.environ.get("LIF_FD", "1024")),
                kadd=int(os.environ.get("LIF_KADD", "6")),
                meng=os.environ.get("LIF_MENG", "pool"),
                xbufs=int(os.environ.get("LIF_XBUFS", "16")),
                pbufs=int(os.environ.get("LIF_PBUFS", "4")),
            )
        elif mode == "pk":
            _compiled = _build_pk(
                spike=os.environ.get("LIF_SPIKE", "act"),
                xbufs=int(os.environ.get("LIF_XBUFS", "12")),
                obufs=int(os.environ.get("LIF_OBUFS", "6")),
            )
        elif mode == "act1":
            _compiled = _build(spike="act1", odt="i8")
        elif mode == "h":
            _compiled = _build_h(
                jm7=int(os.environ.get("LIF_JM7", "4")),
                xbufs=int(os.environ.get("LIF_XBUFS", "12")),
                obufs=int(os.environ.get("LIF_OBUFS", "8")),
            )
        else:
            _compiled = _build()
    return _compiled


def _shard_pe(x: np.ndarray, i: int, fd: int) -> np.ndarray:
    """Core i's shard for pe mode: chunk-major [(c,t,p), fd] rows, with
    x_t pre-scaled by 10^t (v-domain)."""
    nch = SPAT // (P * fd)
    xs = x[i * BS : (i + 1) * BS].reshape(SPAT, T)
    xv = xs * np.asarray(VSCALE, dtype=np.float32)[None, :]
    xv = xv.reshape(nch, P, fd, T).transpose(0, 3, 1, 2)  # [c, t, P, fd]
    return np.ascontiguousarray(xv).reshape(nch * T * P, fd)


def _w_pe() -> np.ndarray:
    import ml_dtypes

    eye = np.eye(P, dtype=np.float32)
    w = np.concatenate([eye * float(1 << t) for t in range(T)], axis=0)
    return w.astype(ml_dtypes.bfloat16)


def _shard_tmajor(x: np.ndarray, i: int) -> np.ndarray:
    """Core i's shard, time-major: [T*NCH*P, FD], row-major over (t, spatial)."""
    xs = x[i * BS : (i + 1) * BS]                   # [BS,C,H,W,T]
    xt = np.moveaxis(xs.reshape(SPAT, T), -1, 0)    # [T, SPAT]
    return np.ascontiguousarray(xt).reshape(ROWS, FD)


def kernel(x: np.ndarray, _trace: bool = False):
    nc = _get_compiled()
    from concourse.bass_utils import run_bass_kernel_spmd

    x = np.asarray(x, dtype=np.float32)
    if _mode() == "pe":
        import os

        fd = int(os.environ.get("LIF_FD", "1024"))
        w = _w_pe()
        in_maps = [
            {"x": _shard_pe(x, i, fd), "w": w} for i in range(NCORES)
        ]
    else:
        in_maps = [{"x": _shard_tmajor(x, i)} for i in range(NCORES)]
    res = run_bass_kernel_spmd(
        nc, in_maps, core_ids=list(range(NCORES)), trace=_trace
    )
    invert = _mode() == "pe"                        # pe packs the keep-mask
    outs = []
    for r in res.results:
        ot = r["o"]
        if ot.size == SPAT:                         # bit-packed u8: bit t = o_t
            if invert:
                ot = np.invert(ot)
            bits = np.unpackbits(
                ot.reshape(-1, 1), axis=1, bitorder="little"
            )[:, :T]
            outs.append(bits.reshape(BS, C, H, W, T).astype(np.float32))
            continue
        if _mode() == "h":                          # u8 masks, mixed polarity
            import os

            jm7 = int(os.environ.get("LIF_JM7", "4"))
            nch = SPAT // (P * FD)
            # rows (t,p), cols (c,j) -> [T, nch, P, FD]
            bits = (ot != 0).reshape(T, P, nch, FD).transpose(0, 2, 1, 3)
            for t in range(T - 1):
                for c in range(nch):
                    if _h_ismul(t, c, nch, jm7):
                        bits[t, c] = ~bits[t, c]
            ot = bits.reshape(T, SPAT).astype(np.float32)
            outs.append(np.moveaxis(ot, 0, -1).reshape(BS, C, H, W, T))
            continue
        if ot.dtype != np.float32:                  # int8 spikes -> f32
            ot = (ot != 0).astype(np.float32)
        ot = ot.reshape(T, SPAT)                    # time-major back to T-last
        outs.append(np.moveaxis(ot, 0, -1).reshape(BS, C, H, W, T))
    out = np.ascontiguousarray(np.concatenate(outs, axis=0))
    return (out, res) if _trace else out



# revision 29
# speedup vs baseline: 5.9210x; 1.1244x over previous
"""LIF spike (leaky integrate-and-fire) forward kernel for Trainium2.

Recurrence over the time axis T=8 of x[64,128,32,32,8] (fp32):
    u_t = TAU * u_{t-1} * (1 - o_{t-1}) + x_t
    o_t = (u_t > VTH)
Data-parallel over the batch dim: 8 NeuronCores x 8 batches each.

Layout: the host transposes each core's shard to time-major [T, spatial]
so that every time-step slice is a contiguous [128, FD] tile (unit-stride
APs for every engine op, contiguous >=1MiB DMAs). Per step the work is:
    o_t  = (u_t > VTH)                 DVE tensor_scalar is_gt -> fp32 out
    w_t  = TAU - TAU*o_t               ScalarE activation Copy(scale,bias),
                                       written in place over o_t after its
                                       store DMA has read it
    u_'  = u_t * w_t                   DVE tensor_tensor mult (in place)
    u_t1 = u_' + x_t1                  DVE tensor_tensor add (in place on
                                       the freshly loaded x tile)
The x tile doubles as the membrane-state buffer, the o tile doubles as the
w buffer, so SBUF holds just two fp32 pools.
"""

import sys

for _p in ("/opt/trn_rl_repo",):
    if _p not in sys.path:
        sys.path.insert(0, _p)

import numpy as np

TAU = 0.1
VTH = 1.5

B, C, H, W, T = 64, 128, 32, 32, 8
NCORES = 8
BS = B // NCORES                      # batches per core
SPAT = BS * C * H * W                 # spatial elems per core per step: 1,048,576
P = 128                               # partitions
FD = 2048                             # free dim per tile
NCH = SPAT // (P * FD)                # spatial chunks per step: 4
ROWS = T * NCH * P                    # dram rows (t-major): 4096
ELEMS = SPAT * T

_compiled = None

# v-domain scaling: v_t = 10^t * u_t kills the TAU multiply (host pre-scales
# x_t by 10^t); thresholds 1.5*10^t are all exact in f32.
VSCALE = [float(10.0**t) for t in range(T)]
VTH_T = [float(1.5 * 10.0**t) for t in range(T)]


def _build_pe(fd: int = 1024, kadd: int = 6, meng: str = "pool",
              xbufs: int = 16, mbufs: int = 8, pbufs: int = 4):
    """v-domain LIF with PE-packed output bytes.

    Recurrence per chunk c (sequential in t):
        v_t = v_{t-1} * m_{t-1} + xs_t      xs_t = 10^t * x_t (host-scaled)
        m_t = (v_t <= 1.5*10^t)             keep-mask, bf16 {0,1}  (DVE ts)
    Packing on the otherwise-idle PE: psum += (2^t I) @ m_t over the 8 steps
    gives byte = sum_t m_t 2^t (exact: bf16 holds {0,1} and 2^t; PSUM is
    f32).  Act copies PSUM -> SBUF u8; host inverts bits (o = NOT m).
    The reset multiply runs on Pool (pure-ish tt), adds split DVE/Pool via
    `kadd` (# adds per chunk on DVE).
    """
    import concourse.bacc as bacc
    import concourse.mybir as mybir
    import concourse.tile as tile

    nch = SPAT // (P * fd)
    nc = bacc.Bacc(
        "TRN2", target_bir_lowering=False, debug=False, num_devices=NCORES
    )
    f32 = mybir.dt.float32
    bf16 = mybir.dt.bfloat16
    u8 = mybir.dt.uint8
    mult = mybir.AluOpType.mult
    add = mybir.AluOpType.add
    is_le = mybir.AluOpType.is_le
    copy_f = mybir.ActivationFunctionType.Copy

    x_d = nc.dram_tensor(
        "x", [nch * T * P, fd], f32, kind="ExternalInput"
    ).ap()
    w_d = nc.dram_tensor("w", [T * P, P], bf16, kind="ExternalInput").ap()
    o_d = nc.dram_tensor("o", [nch * P, fd], u8, kind="ExternalOutput").ap()

    with tile.TileContext(nc) as tc:
        with (
            tc.tile_pool(name="xp", bufs=xbufs) as xp,
            tc.tile_pool(name="mp", bufs=mbufs) as mp,
            tc.tile_pool(name="op", bufs=2) as op_,
            tc.tile_pool(name="wp", bufs=1) as wp,
            tc.psum_pool(name="pp", bufs=pbufs) as pp,
        ):
            wts = []
            for t in range(T):
                wt = wp.tile([P, P], bf16, tag=f"w{t}", name=f"w{t}")
                nc.sync.dma_start(out=wt[:], in_=w_d[t * P : (t + 1) * P, :])
                wts.append(wt)
            for c in range(nch):
                ps = pp.tile([P, fd], f32, name="ps")
                st = None
                mprev = None
                for t in range(T):
                    r0 = (c * T + t) * P
                    xt = xp.tile([P, fd], f32)
                    nc.sync.dma_start(out=xt[:], in_=x_d[r0 : r0 + P, :])
                    if t > 0:
                        # um = v_{t-1} * m_{t-1}  (in place on state tile)
                        me = nc.gpsimd if meng == "pool" else nc.vector
                        me.tensor_tensor(
                            out=st[:], in0=st[:], in1=mprev[:], op=mult
                        )
                        # v_t = um + xs_t  (in place on the x tile)
                        ae = nc.vector if t <= kadd else nc.gpsimd
                        ae.tensor_tensor(
                            out=xt[:], in0=st[:], in1=xt[:], op=add
                        )
                    st = xt
                    m = mp.tile([P, fd], bf16)
                    nc.vector.tensor_scalar(
                        m[:], st[:], VTH_T[t], None, is_le
                    )
                    # PSUM bank limit: <=512 f32 out columns per matmul
                    for h in range(fd // 512):
                        sl = slice(h * 512, (h + 1) * 512)
                        nc.tensor.matmul(
                            ps[:, sl], wts[t][:], m[:, sl],
                            start=(t == 0), stop=(t == T - 1),
                        )
                    mprev = m
                ot = op_.tile([P, fd], u8)
                nc.scalar.activation(ot[:], ps[:], copy_f)
                nc.sync.dma_start(
                    out=o_d[c * P : (c + 1) * P, :], in_=ot[:]
                )
    nc.compile()
    return nc


def _h_ismul(t: int, c: int, nch: int, jm7: int) -> bool:
    """Static per-quantum choice: True = reset via Pool multiply (Act emits
    the keep-mask), False = reset via DVE copy_predicated (Act emits the
    spike-mask).  t is the step whose mask this is (0..T-2)."""
    return (t * nch + c) % 7 < jm7


def _build_h(fd: int = FD, jm7: int = 4, xbufs: int = 12, obufs: int = 8,
             mulf: bool = False):
    """i8-out hybrid: per step the Act engine emits one u8 {0,1} mask tile
    (spike- or keep-oriented), which is both the reset selector and the DMA'd
    output byte (host re-inverts keep-oriented blocks).

    Per step t>0, chunk c (28 update quanta):
      cp path:   u' = 0 where o_prev      DVE copy_predicated (2.4us/q)
      mul path:  u' = u * m_prev          Pool mixed u8*f32 tt (6.0us/q)
      then       u_t = TAU*u' + x_t       DVE stt (2.3us/q)
      mask       Act Sign(+-(u-VTH))      2.0us/q
    jm7/7 of quanta take the mul path, balancing DVE ~93us / Pool ~96us
    under the 42MB DMA wall (~118us @ 356GB/s measured).
    """
    import concourse.bacc as bacc
    import concourse.mybir as mybir
    import concourse.tile as tile

    nch = SPAT // (P * fd)
    nc = bacc.Bacc(
        "TRN2", target_bir_lowering=False, debug=False, num_devices=NCORES
    )
    f32 = mybir.dt.float32
    u8 = mybir.dt.uint8
    mult = mybir.AluOpType.mult
    add = mybir.AluOpType.add
    is_le = mybir.AluOpType.is_le
    sign_f = mybir.ActivationFunctionType.Sign

    x_d = nc.dram_tensor("x", [T * nch * P, fd], f32, kind="ExternalInput").ap()
    # one fat row block per step: 8KB rows for efficient output DMA
    o_d = nc.dram_tensor("o", [T * P, nch * fd], u8, kind="ExternalOutput").ap()

    with tile.TileContext(nc) as tc:
        with (
            tc.tile_pool(name="xp", bufs=xbufs) as xp,
            tc.tile_pool(name="op", bufs=obufs) as op_,
            tc.tile_pool(name="mp", bufs=4) as mp,
            tc.tile_pool(name="cp", bufs=1) as cp,
        ):
            zero = cp.tile([P, fd], f32, tag="zero")
            nc.gpsimd.memset(zero[:], 0.0)
            nvth = cp.tile([P, 1], f32, tag="nvth")
            nc.gpsimd.memset(nvth[:], -VTH)
            pvth = cp.tile([P, 1], f32, tag="pvth")
            nc.gpsimd.memset(pvth[:], VTH)

            st = [None] * nch    # state tile per chunk
            mk = [None] * nch    # mask AP (slice of the staging tile)
            for t in range(T):
                ot = op_.tile([P, nch * fd], u8)
                for c in range(nch):
                    r0 = (t * nch + c) * P
                    xt = xp.tile([P, fd], f32)
                    nc.sync.dma_start(out=xt[:], in_=x_d[r0 : r0 + P, :])
                    if t > 0:
                        if _h_ismul(t - 1, c, nch, jm7):
                            if mulf:
                                # pure-f32: u' = u * m_f32  (Pool, no u8)
                                nc.gpsimd.tensor_tensor(
                                    out=st[c][:], in0=st[c][:], in1=mk[c],
                                    op=mult,
                                )
                            else:
                                # keep-mask: u' = u * m  (mixed u8*f32, Pool)
                                nc.gpsimd.tensor_tensor(
                                    out=st[c][:], in0=st[c][:], in1=mk[c],
                                    op=mult,
                                )
                        else:
                            # spike-mask: zero u where spiked (DVE)
                            nc.vector.copy_predicated(
                                out=st[c][:], mask=mk[c], data=zero[:]
                            )
                        # u_t = TAU*u' + x_t  (in place on x tile, DVE)
                        nc.vector.scalar_tensor_tensor(
                            out=xt[:], in0=st[c][:], scalar=TAU, in1=xt[:],
                            op0=mult, op1=add,
                        )
                    st[c] = xt
                    o = ot[:, c * fd : (c + 1) * fd]
                    if mulf:
                        # output always spike-oriented
                        nc.scalar.activation(
                            o, st[c][:], sign_f, bias=nvth[:]
                        )
                        if t < T - 1 and _h_ismul(t, c, nch, jm7):
                            # f32 keep-mask for the Pool multiply (DVE ts 2x)
                            mf = mp.tile([P, fd], f32, name="mf")
                            nc.vector.tensor_scalar(
                                mf[:], st[c][:], VTH, None, is_le
                            )
                            mk[c] = mf[:]
                        else:
                            mk[c] = o
                    elif t < T - 1 and _h_ismul(t, c, nch, jm7):
                        # m = sign(VTH - u) -> u8 {0,1}: keep-mask
                        nc.scalar.activation(
                            o, st[c][:], sign_f, bias=pvth[:], scale=-1.0
                        )
                        mk[c] = o
                    else:
                        # o = sign(u - VTH) -> u8 {0,1}: spike-mask
                        nc.scalar.activation(
                            o, st[c][:], sign_f, bias=nvth[:]
                        )
                        mk[c] = o
                nc.sync.dma_start(
                    out=o_d[t * P : (t + 1) * P, :], in_=ot[:]
                )
    nc.compile()
    return nc


def _build_pk(spike: str = "act", xbufs: int = 12, obufs: int = 6, fd: int = FD):
    """Bit-packed output variant: one u8 byte per spatial element holding all
    T=8 spikes (bit t = o_t), cutting output HBM traffic 32x vs f32.

    Per time step t, per [P, fd] chunk c (engine assignment in parens):
      decay   u_t = TAU*u'_{t-1} + x_t        stt, in place on x tile  (Pool)
      spike   o_t = (u_t > VTH) as u8 {0,1}   (Act: Sign(u-VTH) -> u8, the
                                               -1 saturating to 0; or DVE/Pool
                                               tensor_scalar is_gt)
      pack    acc += o_t << t                 stt, acc is the u8 out tile (DVE)
      reset   u'_t = 0 where o_t              copy_predicated, mask=o_t (DVE)
    """
    import concourse.bacc as bacc
    import concourse.mybir as mybir
    import concourse.tile as tile

    nch = SPAT // (P * fd)
    nc = bacc.Bacc(
        "TRN2", target_bir_lowering=False, debug=False, num_devices=NCORES
    )
    f32 = mybir.dt.float32
    u8 = mybir.dt.uint8
    mult = mybir.AluOpType.mult
    add = mybir.AluOpType.add
    is_gt = mybir.AluOpType.is_gt
    is_le = mybir.AluOpType.is_le
    sign_f = mybir.ActivationFunctionType.Sign

    x_d = nc.dram_tensor("x", [T * nch * P, fd], f32, kind="ExternalInput").ap()
    o_d = nc.dram_tensor("o", [nch * P, fd], u8, kind="ExternalOutput").ap()

    with tile.TileContext(nc) as tc:
        with (
            tc.tile_pool(name="xp", bufs=xbufs) as xp,
            tc.tile_pool(name="op", bufs=obufs) as op_,
            tc.tile_pool(name="cp", bufs=1) as cp,
        ):
            zero = None
            nvth = None
            if spike != "ts":
                zero = cp.tile([P, fd], f32, tag="zero")
                nc.gpsimd.memset(zero[:], 0.0)
                nvth = cp.tile([P, 1], f32, tag="nvth")
                nc.gpsimd.memset(nvth[:], -VTH)
            # f32 accumulator (Pool can't do u8+u8 adds); u8 out tile is
            # written once by the final t=7 pack op.
            acc = [
                cp.tile([P, fd], f32, tag=f"acc{c}", name=f"acc{c}")
                for c in range(nch)
            ]
            out8 = [
                cp.tile([P, fd], u8, tag=f"out{c}", name=f"out{c}")
                for c in range(nch)
            ]
            st = [None] * nch
            for t in range(T):
                for c in range(nch):
                    r0 = (t * nch + c) * P
                    xt = xp.tile([P, fd], f32)
                    nc.sync.dma_start(out=xt[:], in_=x_d[r0 : r0 + P, :])
                    if t > 0:
                        # u_t = TAU*u' + x_t  (in place on the x tile).
                        # stt is DVE-only on v3 (Pool rejects TensorScalarPtr
                        # in the stt form).
                        nc.vector.scalar_tensor_tensor(
                            out=xt[:], in0=st[c][:], scalar=TAU, in1=xt[:],
                            op0=mult, op1=add,
                        )
                    st[c] = xt
                    if spike == "ts":
                        # mask-free: weighted spike + gated state, all-DVE
                        ws = op_.tile([P, fd], f32, name="wsf")
                        nc.vector.tensor_scalar(
                            ws[:], st[c][:], VTH, float(1 << t), is_gt, mult
                        )
                        if t == 0:
                            nc.vector.tensor_scalar(
                                acc[c][:], ws[:], 1.0, None, mult
                            )
                        else:
                            dst = out8[c] if t == T - 1 else acc[c]
                            nc.vector.tensor_tensor(
                                out=dst[:], in0=ws[:], in1=acc[c][:], op=add
                            )
                        if t < T - 1:
                            # u'' = (u <= VTH) * u   (kills spiked state)
                            nc.vector.scalar_tensor_tensor(
                                out=st[c][:], in0=st[c][:], scalar=VTH,
                                in1=st[c][:], op0=is_le, op1=mult,
                            )
                    else:
                        o = op_.tile([P, fd], u8)
                        if spike == "act":
                            # o = sign(u - VTH) -> u8: -1 saturates to 0
                            nc.scalar.activation(
                                o[:], st[c][:], sign_f, bias=nvth[:]
                            )
                        else:
                            eng = nc.vector if (t + c) % 2 else nc.gpsimd
                            eng.tensor_scalar(o[:], st[c][:], VTH, None, is_gt)
                        # pack: acc (f32) += o << t.  DVE already carries
                        # decay+reset (57us floor), so t<=5 pack goes to the
                        # otherwise-idle Pool engine (u8 ts, then the legal
                        # mixed u8+f32 tt add); t=6,7 are single DVE stt ops,
                        # t=7 writing the final u8 byte.
                        if t == 0:
                            nc.gpsimd.tensor_scalar(
                                acc[c][:], o[:], 1.0, None, mult
                            )
                        elif t <= 5:
                            ws = op_.tile([P, fd], u8, name="ws")
                            nc.gpsimd.tensor_scalar(
                                ws[:], o[:], float(1 << t), None, mult
                            )
                            nc.gpsimd.tensor_tensor(
                                out=acc[c][:], in0=ws[:], in1=acc[c][:],
                                op=add,
                            )
                        else:
                            dst = out8[c] if t == T - 1 else acc[c]
                            nc.vector.scalar_tensor_tensor(
                                out=dst[:], in0=o[:], scalar=float(1 << t),
                                in1=acc[c][:], op0=mult, op1=add,
                            )
                        if t < T - 1:
                            nc.vector.copy_predicated(
                                out=st[c][:], mask=o[:], data=zero[:]
                            )
                    if t == T - 1:
                        nc.sync.dma_start(
                            out=o_d[c * P : (c + 1) * P, :], in_=out8[c][:]
                        )
    nc.compile()
    return nc


def _build(
    reps: int = 1,
    mode: str = "full",
    bufs=(10, 10),
    fd=FD,
    odt: str = "f32",
    ger: str = "v",
    mer: str = "v",
    spike: str = "dve",
    pack: bool = False,
    ib: int = 2,
):
    import contextlib

    import concourse.bacc as bacc
    import concourse.mybir as mybir
    import concourse.tile as tile

    nch = SPAT // (P * fd)
    nc = bacc.Bacc(
        "TRN2",
        target_bir_lowering=False,
        debug=False,
        num_devices=NCORES,
    )
    f32 = mybir.dt.float32
    odtype = f32 if odt == "f32" else mybir.dt.int8
    if pack:
        # in rows (t, cg, p) cols (half, j); out rows (t, p) cols (c, j)
        x_d = nc.dram_tensor(
            "x", [T * (nch // ib) * P, ib * fd], f32, kind="ExternalInput"
        ).ap()
        o_d = nc.dram_tensor(
            "o", [T * P, nch * fd], mybir.dt.int8, kind="ExternalOutput"
        ).ap()
    else:
        x_d = nc.dram_tensor(
            "x", [T * nch * P, fd], f32, kind="ExternalInput"
        ).ap()
        o_d = nc.dram_tensor(
            "o", [T * nch * P, fd], odtype, kind="ExternalOutput"
        ).ap()

    with tile.TileContext(nc) as tc:
        with (
            tc.tile_pool(name="xp", bufs=bufs[0]) as xp,
            tc.tile_pool(name="op", bufs=bufs[1]) as op_,
            tc.tile_pool(name="wp", bufs=6) as wp,
            tc.tile_pool(name="cp", bufs=1) as cp,
        ):
            rep_ctx = (
                tc.For_i(0, reps, 1) if reps > 1 else contextlib.nullcontext()
            )
            with rep_ctx:
                if pack:
                    _emit_packed(nc, xp, op_, cp, x_d, o_d, mybir, mode,
                                 fd, nch, ib)
                else:
                    _emit(nc, xp, op_, wp, cp, x_d, o_d, mybir, mode, fd,
                          nch, odt, ger, mer, spike)
    nc.compile()
    return nc


def _emit_packed(nc, xp, op_, cp, x_d, o_d, mybir, mode, fd, nch, ib):
    """act1-spike i8-out variant with batched DMAs.

    Input tiles span `ib` chunks (one contiguous DMA each); output tiles
    span all `nch` chunks of a step (one contiguous DMA per step).
    """
    f32 = mybir.dt.float32
    i8 = mybir.dt.int8
    mult = mybir.AluOpType.mult
    add = mybir.AluOpType.add
    relu_f = mybir.ActivationFunctionType.Relu
    dma, compute = mode in ("full", "dma"), mode in ("full", "compute")
    ng = nch // ib

    zero = cp.tile([P, fd], f32, tag="zero")
    nc.gpsimd.memset(zero[:], 0.0)
    nvthbig = cp.tile([P, 1], f32, tag="nvthbig")
    nc.gpsimd.memset(nvthbig[:], -VTH * 1e9)

    u = [None] * nch       # AP slice holding u_t per chunk
    o_prev = [None] * nch  # AP slice of o_{t-1} per chunk
    for t in range(T):
        xts = []
        for g in range(ng):
            xt = xp.tile([P, ib * fd], f32)
            if dma:
                r0 = (t * ng + g) * P
                nc.sync.dma_start(out=xt[:], in_=x_d[r0 : r0 + P, :])
            elif t == 0:
                nc.gpsimd.memset(xt[:], 0.25)
            xts.append(xt)
        ot = op_.tile([P, nch * fd], i8)
        for c in range(nch):
            g, h = c // ib, c % ib
            xs = xts[g][:, h * fd : (h + 1) * fd]
            if compute:
                if t > 0:
                    # reset where previous step spiked
                    nc.vector.copy_predicated(
                        out=u[c], mask=o_prev[c], data=zero[:]
                    )
                    # u_t = TAU*u_masked + x_t  (in place on x slice)
                    nc.vector.scalar_tensor_tensor(
                        out=xs, in0=u[c], scalar=TAU, in1=xs,
                        op0=mult, op1=add,
                    )
                u[c] = xs
                # o8 = sat_i8(relu(1e9*u - 1e9*VTH)): nonzero iff spike
                nc.scalar.activation(
                    ot[:, c * fd : (c + 1) * fd], u[c], relu_f,
                    bias=nvthbig[:], scale=1e9,
                )
                o_prev[c] = ot[:, c * fd : (c + 1) * fd]
        if not compute:
            nc.gpsimd.memset(ot[:, :1], 1)
        if dma:
            nc.sync.dma_start(out=o_d[t * P : (t + 1) * P, :], in_=ot[:])


def _emit(nc, xp, op_, wp, cp, x_d, o_d, mybir, mode, fd, nch, odt, ger, mer,
          spike="dve"):
    f32 = mybir.dt.float32
    mult = mybir.AluOpType.mult
    add = mybir.AluOpType.add
    is_gt = mybir.AluOpType.is_gt
    copy_f = mybir.ActivationFunctionType.Copy
    dma, compute = mode in ("full", "dma"), mode in ("full", "compute")
    odtype = f32 if odt == "f32" else mybir.dt.int8
    geng = nc.vector if ger == "v" else nc.gpsimd
    meng = nc.vector if mer == "v" else nc.gpsimd

    i8 = mybir.dt.int8
    relu_f = mybir.ActivationFunctionType.Relu
    sign_f = mybir.ActivationFunctionType.Sign

    o8c = None
    if mode == "dma" and odt == "i8":
        o8c = cp.tile([P, fd], i8, tag="o8c")
        nc.gpsimd.memset(o8c[:], 1)
    if spike in ("act", "act1"):
        assert odt == "i8"
        zero = cp.tile([P, fd], f32, tag="zero")
        nc.gpsimd.memset(zero[:], 0.0)
        nvth = cp.tile([P, 1], f32, tag="nvth")
        nc.gpsimd.memset(nvth[:], -VTH)
        nvthbig = cp.tile([P, 1], f32, tag="nvthbig")
        nc.gpsimd.memset(nvthbig[:], -VTH * 1e9)

    u = [None] * nch       # tile holding u_t per chunk
    o_prev = [None] * nch  # tile holding o_{t-1} per chunk
    for t in range(T):
        for c in range(nch):
            r0 = (t * nch + c) * P
            xt = xp.tile([P, fd], f32)
            if dma:
                nc.sync.dma_start(out=xt[:], in_=x_d[r0 : r0 + P, :])
            elif t == 0:
                nc.gpsimd.memset(xt[:], 0.25)
            if compute and spike in ("act", "act1"):
                if t > 0:
                    o = o_prev[c]
                    # reset where previous step spiked
                    nc.vector.copy_predicated(
                        out=u[c][:], mask=o[:], data=zero[:]
                    )
                    # u_t = TAU*u_masked + x_t  (in place on x tile)
                    nc.vector.scalar_tensor_tensor(
                        out=xt[:], in0=u[c][:], scalar=TAU, in1=xt[:],
                        op0=mult, op1=add,
                    )
                u[c] = xt
                ot = op_.tile([P, fd], i8)
                if spike == "act1":
                    # o8 = sat_i8(relu(1e9*u - 1e9*VTH)): nonzero iff spike.
                    # int8 conversion saturates at 127 (verified on HW), and
                    # |u-VTH| >= 1 ulp(1.5) so the *1e9 rounding never
                    # crosses zero.
                    nc.scalar.activation(
                        ot[:], u[c][:], relu_f, bias=nvthbig[:], scale=1e9
                    )
                else:
                    # spike on ScalarE: sg = sign(u - VTH); o = relu(sg)
                    sg = wp.tile([P, fd], f32, tag="sg")
                    nc.scalar.activation(sg[:], u[c][:], sign_f, bias=nvth[:])
                    nc.scalar.activation(ot[:], sg[:], relu_f)
                o_prev[c] = ot
            elif compute:
                if t == 0:
                    u[c] = xt
                else:
                    o = o_prev[c]
                    if odt == "f32":
                        # w <- TAU - TAU*o  (in place over o after its store)
                        w = o
                        nc.scalar.activation(
                            w[:], o[:], copy_f, bias=TAU, scale=-TAU
                        )
                    else:
                        w = wp.tile([P, fd], f32)
                        nc.scalar.activation(
                            w[:], o[:], copy_f, bias=TAU, scale=-TAU
                        )
                    # u_masked = u_{t-1} * w   (in place)
                    meng.tensor_tensor(
                        out=u[c][:], in0=u[c][:], in1=w[:], op=mult
                    )
                    # u_t = u_masked + x_t    (in place on x tile)
                    nc.vector.tensor_tensor(
                        out=xt[:], in0=u[c][:], in1=xt[:], op=add
                    )
                    u[c] = xt
                ot = op_.tile([P, fd], odtype)
                geng.tensor_scalar(ot[:], u[c][:], VTH, None, is_gt)
                o_prev[c] = ot
            else:
                ot = o8c if o8c is not None else xt
            if dma:
                nc.sync.dma_start(out=o_d[r0 : r0 + P, :], in_=ot[:])


def _mode():
    import os

    return os.environ.get("LIF_MODE", "pe")


def _get_compiled():
    global _compiled
    if _compiled is None:
        import os

        mode = _mode()
        if mode == "pe":
            _compiled = _build_pe(
                fd=int(os.environ.get("LIF_FD", "1024")),
                kadd=int(os.environ.get("LIF_KADD", "6")),
                meng=os.environ.get("LIF_MENG", "pool"),
                xbufs=int(os.environ.get("LIF_XBUFS", "16")),
                pbufs=int(os.environ.get("LIF_PBUFS", "4")),
            )
        elif mode == "pk":
            _compiled = _build_pk(
                spike=os.environ.get("LIF_SPIKE", "act"),
                xbufs=int(os.environ.get("LIF_XBUFS", "12")),
                obufs=int(os.environ.get("LIF_OBUFS", "6")),
            )
        elif mode == "act1":
            _compiled = _build(spike="act1", odt="i8")
        elif mode == "h":
            _compiled = _build_h(
                jm7=int(os.environ.get("LIF_JM7", "4")),
                xbufs=int(os.environ.get("LIF_XBUFS", "12")),
                obufs=int(os.environ.get("LIF_OBUFS", "8")),
                mulf=os.environ.get("LIF_MULF", "0") == "1",
            )
        else:
            _compiled = _build()
    return _compiled


def _shard_pe(x: np.ndarray, i: int, fd: int) -> np.ndarray:
    """Core i's shard for pe mode: chunk-major [(c,t,p), fd] rows, with
    x_t pre-scaled by 10^t (v-domain)."""
    nch = SPAT // (P * fd)
    xs = x[i * BS : (i + 1) * BS].reshape(SPAT, T)
    xv = xs * np.asarray(VSCALE, dtype=np.float32)[None, :]
    xv = xv.reshape(nch, P, fd, T).transpose(0, 3, 1, 2)  # [c, t, P, fd]
    return np.ascontiguousarray(xv).reshape(nch * T * P, fd)


def _w_pe() -> np.ndarray:
    import ml_dtypes

    eye = np.eye(P, dtype=np.float32)
    w = np.concatenate([eye * float(1 << t) for t in range(T)], axis=0)
    return w.astype(ml_dtypes.bfloat16)


def _shard_tmajor(x: np.ndarray, i: int) -> np.ndarray:
    """Core i's shard, time-major: [T*NCH*P, FD], row-major over (t, spatial)."""
    xs = x[i * BS : (i + 1) * BS]                   # [BS,C,H,W,T]
    xt = np.moveaxis(xs.reshape(SPAT, T), -1, 0)    # [T, SPAT]
    return np.ascontiguousarray(xt).reshape(ROWS, FD)


def kernel(x: np.ndarray, _trace: bool = False):
    nc = _get_compiled()
    from concourse.bass_utils import run_bass_kernel_spmd

    x = np.asarray(x, dtype=np.float32)
    if _mode() == "pe":
        import os

        fd = int(os.environ.get("LIF_FD", "1024"))
        w = _w_pe()
        in_maps = [
            {"x": _shard_pe(x, i, fd), "w": w} for i in range(NCORES)
        ]
    else:
        in_maps = [{"x": _shard_tmajor(x, i)} for i in range(NCORES)]
    res = run_bass_kernel_spmd(
        nc, in_maps, core_ids=list(range(NCORES)), trace=_trace
    )
    invert = _mode() == "pe"                        # pe packs the keep-mask
    outs = []
    for r in res.results:
        ot = r["o"]
        if ot.size == SPAT:                         # bit-packed u8: bit t = o_t
            if invert:
                ot = np.invert(ot)
            bits = np.unpackbits(
                ot.reshape(-1, 1), axis=1, bitorder="little"
            )[:, :T]
            outs.append(bits.reshape(BS, C, H, W, T).astype(np.float32))
            continue
        if _mode() == "h":                          # u8 masks, mixed polarity
            import os

            jm7 = int(os.environ.get("LIF_JM7", "4"))
            mulf = os.environ.get("LIF_MULF", "0") == "1"
            nch = SPAT // (P * FD)
            # rows (t,p), cols (c,j) -> [T, nch, P, FD]
            bits = (ot != 0).reshape(T, P, nch, FD).transpose(0, 2, 1, 3)
            if not mulf:                            # keep-mask blocks invert
                for t in range(T - 1):
                    for c in range(nch):
                        if _h_ismul(t, c, nch, jm7):
                            bits[t, c] = ~bits[t, c]
            ot = bits.reshape(T, SPAT).astype(np.float32)
            outs.append(np.moveaxis(ot, 0, -1).reshape(BS, C, H, W, T))
            continue
        if ot.dtype != np.float32:                  # int8 spikes -> f32
            ot = (ot != 0).astype(np.float32)
        ot = ot.reshape(T, SPAT)                    # time-major back to T-last
        outs.append(np.moveaxis(ot, 0, -1).reshape(BS, C, H, W, T))
    out = np.ascontiguousarray(np.concatenate(outs, axis=0))
    return (out, res) if _trace else out



# revision 32
# speedup vs baseline: 5.9355x; 1.0025x over previous
"""LIF spike (leaky integrate-and-fire) forward kernel for Trainium2.

Recurrence over the time axis T=8 of x[64,128,32,32,8] (fp32):
    u_t = TAU * u_{t-1} * (1 - o_{t-1}) + x_t
    o_t = (u_t > VTH)
Data-parallel over the batch dim: 8 NeuronCores x 8 batches each.

Layout: the host transposes each core's shard to time-major [T, spatial]
so that every time-step slice is a contiguous [128, FD] tile (unit-stride
APs for every engine op, contiguous >=1MiB DMAs). Per step the work is:
    o_t  = (u_t > VTH)                 DVE tensor_scalar is_gt -> fp32 out
    w_t  = TAU - TAU*o_t               ScalarE activation Copy(scale,bias),
                                       written in place over o_t after its
                                       store DMA has read it
    u_'  = u_t * w_t                   DVE tensor_tensor mult (in place)
    u_t1 = u_' + x_t1                  DVE tensor_tensor add (in place on
                                       the freshly loaded x tile)
The x tile doubles as the membrane-state buffer, the o tile doubles as the
w buffer, so SBUF holds just two fp32 pools.
"""

import sys

for _p in ("/opt/trn_rl_repo",):
    if _p not in sys.path:
        sys.path.insert(0, _p)

import numpy as np

TAU = 0.1
VTH = 1.5

B, C, H, W, T = 64, 128, 32, 32, 8
NCORES = 8
BS = B // NCORES                      # batches per core
SPAT = BS * C * H * W                 # spatial elems per core per step: 1,048,576
P = 128                               # partitions
FD = 2048                             # free dim per tile
NCH = SPAT // (P * FD)                # spatial chunks per step: 4
ROWS = T * NCH * P                    # dram rows (t-major): 4096
ELEMS = SPAT * T

_compiled = None

# v-domain scaling: v_t = 10^t * u_t kills the TAU multiply (host pre-scales
# x_t by 10^t); thresholds 1.5*10^t are all exact in f32.
VSCALE = [float(10.0**t) for t in range(T)]
VTH_T = [float(1.5 * 10.0**t) for t in range(T)]


def _build_pe(fd: int = 1024, kadd: int = 6, meng: str = "pool",
              xbufs: int = 16, mbufs: int = 8, pbufs: int = 4):
    """v-domain LIF with PE-packed output bytes.

    Recurrence per chunk c (sequential in t):
        v_t = v_{t-1} * m_{t-1} + xs_t      xs_t = 10^t * x_t (host-scaled)
        m_t = (v_t <= 1.5*10^t)             keep-mask, bf16 {0,1}  (DVE ts)
    Packing on the otherwise-idle PE: psum += (2^t I) @ m_t over the 8 steps
    gives byte = sum_t m_t 2^t (exact: bf16 holds {0,1} and 2^t; PSUM is
    f32).  Act copies PSUM -> SBUF u8; host inverts bits (o = NOT m).
    The reset multiply runs on Pool (pure-ish tt), adds split DVE/Pool via
    `kadd` (# adds per chunk on DVE).
    """
    import concourse.bacc as bacc
    import concourse.mybir as mybir
    import concourse.tile as tile

    nch = SPAT // (P * fd)
    nc = bacc.Bacc(
        "TRN2", target_bir_lowering=False, debug=False, num_devices=NCORES
    )
    f32 = mybir.dt.float32
    bf16 = mybir.dt.bfloat16
    u8 = mybir.dt.uint8
    mult = mybir.AluOpType.mult
    add = mybir.AluOpType.add
    is_le = mybir.AluOpType.is_le
    copy_f = mybir.ActivationFunctionType.Copy

    x_d = nc.dram_tensor(
        "x", [nch * T * P, fd], f32, kind="ExternalInput"
    ).ap()
    w_d = nc.dram_tensor("w", [T * P, P], bf16, kind="ExternalInput").ap()
    o_d = nc.dram_tensor("o", [nch * P, fd], u8, kind="ExternalOutput").ap()

    with tile.TileContext(nc) as tc:
        with (
            tc.tile_pool(name="xp", bufs=xbufs) as xp,
            tc.tile_pool(name="mp", bufs=mbufs) as mp,
            tc.tile_pool(name="op", bufs=2) as op_,
            tc.tile_pool(name="wp", bufs=1) as wp,
            tc.psum_pool(name="pp", bufs=pbufs) as pp,
        ):
            wts = []
            for t in range(T):
                wt = wp.tile([P, P], bf16, tag=f"w{t}", name=f"w{t}")
                nc.sync.dma_start(out=wt[:], in_=w_d[t * P : (t + 1) * P, :])
                wts.append(wt)
            for c in range(nch):
                ps = pp.tile([P, fd], f32, name="ps")
                st = None
                mprev = None
                for t in range(T):
                    r0 = (c * T + t) * P
                    xt = xp.tile([P, fd], f32)
                    nc.sync.dma_start(out=xt[:], in_=x_d[r0 : r0 + P, :])
                    if t > 0:
                        # um = v_{t-1} * m_{t-1}  (in place on state tile)
                        me = nc.gpsimd if meng == "pool" else nc.vector
                        me.tensor_tensor(
                            out=st[:], in0=st[:], in1=mprev[:], op=mult
                        )
                        # v_t = um + xs_t  (in place on the x tile)
                        ae = nc.vector if t <= kadd else nc.gpsimd
                        ae.tensor_tensor(
                            out=xt[:], in0=st[:], in1=xt[:], op=add
                        )
                    st = xt
                    m = mp.tile([P, fd], bf16)
                    nc.vector.tensor_scalar(
                        m[:], st[:], VTH_T[t], None, is_le
                    )
                    # PSUM bank limit: <=512 f32 out columns per matmul
                    for h in range(fd // 512):
                        sl = slice(h * 512, (h + 1) * 512)
                        nc.tensor.matmul(
                            ps[:, sl], wts[t][:], m[:, sl],
                            start=(t == 0), stop=(t == T - 1),
                        )
                    mprev = m
                ot = op_.tile([P, fd], u8)
                nc.scalar.activation(ot[:], ps[:], copy_f)
                nc.sync.dma_start(
                    out=o_d[c * P : (c + 1) * P, :], in_=ot[:]
                )
    nc.compile()
    return nc


def _h_ismul(t: int, c: int, nch: int, jm7: int) -> bool:
    """Static per-quantum choice: True = reset via Pool multiply (Act emits
    the keep-mask), False = reset via DVE copy_predicated (Act emits the
    spike-mask).  t is the step whose mask this is (0..T-2)."""
    return (t * nch + c) % 7 < jm7


def _build_h(fd: int = FD, jm7: int = 4, xbufs: int = 12, obufs: int = 8,
             mulf: bool = False):
    """i8-out hybrid: per step the Act engine emits one u8 {0,1} mask tile
    (spike- or keep-oriented), which is both the reset selector and the DMA'd
    output byte (host re-inverts keep-oriented blocks).

    Per step t>0, chunk c (28 update quanta):
      cp path:   u' = 0 where o_prev      DVE copy_predicated (2.4us/q)
      mul path:  u' = u * m_prev          Pool mixed u8*f32 tt (6.0us/q)
      then       u_t = TAU*u' + x_t       DVE stt (2.3us/q)
      mask       Act Sign(+-(u-VTH))      2.0us/q
    jm7/7 of quanta take the mul path, balancing DVE ~93us / Pool ~96us
    under the 42MB DMA wall (~118us @ 356GB/s measured).
    """
    import concourse.bacc as bacc
    import concourse.mybir as mybir
    import concourse.tile as tile

    nch = SPAT // (P * fd)
    nc = bacc.Bacc(
        "TRN2", target_bir_lowering=False, debug=False, num_devices=NCORES
    )
    f32 = mybir.dt.float32
    u8 = mybir.dt.uint8
    mult = mybir.AluOpType.mult
    add = mybir.AluOpType.add
    is_le = mybir.AluOpType.is_le
    sign_f = mybir.ActivationFunctionType.Sign

    x_d = nc.dram_tensor("x", [T * nch * P, fd], f32, kind="ExternalInput").ap()
    # one fat row block per step: 8KB rows for efficient output DMA
    o_d = nc.dram_tensor("o", [T * P, nch * fd], u8, kind="ExternalOutput").ap()

    with tile.TileContext(nc) as tc:
        with (
            tc.tile_pool(name="xp", bufs=xbufs) as xp,
            tc.tile_pool(name="op", bufs=obufs) as op_,
            tc.tile_pool(name="mp", bufs=4) as mp,
            tc.tile_pool(name="cp", bufs=1) as cp,
        ):
            zero = cp.tile([P, fd], f32, tag="zero")
            nc.gpsimd.memset(zero[:], 0.0)
            nvth = cp.tile([P, 1], f32, tag="nvth")
            nc.gpsimd.memset(nvth[:], -VTH)
            pvth = cp.tile([P, 1], f32, tag="pvth")
            nc.gpsimd.memset(pvth[:], VTH)

            st = [None] * nch    # state tile per chunk
            mk = [None] * nch    # mask AP (slice of the staging tile)
            # quantum order: steps 0/1 interleaved per chunk so the first
            # stt only waits on two 1MiB loads, then t-major
            order = []
            for c in range(nch):
                order += [(0, c), (1, c)]
            for t in range(2, T):
                order += [(t, c) for c in range(nch)]
            ots = {}
            for t, c in order:
                if c == 0:
                    ots[t] = op_.tile([P, nch * fd], u8, name="ot")
                ot = ots[t]
                if True:
                    r0 = (t * nch + c) * P
                    xt = xp.tile([P, fd], f32)
                    nc.sync.dma_start(out=xt[:], in_=x_d[r0 : r0 + P, :])
                    if t > 0:
                        if _h_ismul(t - 1, c, nch, jm7):
                            if mulf:
                                # pure-f32: u' = u * m_f32  (Pool, no u8)
                                nc.gpsimd.tensor_tensor(
                                    out=st[c][:], in0=st[c][:], in1=mk[c],
                                    op=mult,
                                )
                            else:
                                # keep-mask: u' = u * m  (mixed u8*f32, Pool)
                                nc.gpsimd.tensor_tensor(
                                    out=st[c][:], in0=st[c][:], in1=mk[c],
                                    op=mult,
                                )
                        else:
                            # spike-mask: zero u where spiked (DVE)
                            nc.vector.copy_predicated(
                                out=st[c][:], mask=mk[c], data=zero[:]
                            )
                        # u_t = TAU*u' + x_t  (in place on x tile, DVE)
                        nc.vector.scalar_tensor_tensor(
                            out=xt[:], in0=st[c][:], scalar=TAU, in1=xt[:],
                            op0=mult, op1=add,
                        )
                    st[c] = xt
                    o = ot[:, c * fd : (c + 1) * fd]
                    if mulf:
                        # output always spike-oriented
                        nc.scalar.activation(
                            o, st[c][:], sign_f, bias=nvth[:]
                        )
                        if t < T - 1 and _h_ismul(t, c, nch, jm7):
                            # f32 keep-mask for the Pool multiply (DVE ts 2x)
                            mf = mp.tile([P, fd], f32, name="mf")
                            nc.vector.tensor_scalar(
                                mf[:], st[c][:], VTH, None, is_le
                            )
                            mk[c] = mf[:]
                        else:
                            mk[c] = o
                    elif t < T - 1 and _h_ismul(t, c, nch, jm7):
                        # m = sign(VTH - u) -> u8 {0,1}: keep-mask
                        nc.scalar.activation(
                            o, st[c][:], sign_f, bias=pvth[:], scale=-1.0
                        )
                        mk[c] = o
                    else:
                        # o = sign(u - VTH) -> u8 {0,1}: spike-mask
                        nc.scalar.activation(
                            o, st[c][:], sign_f, bias=nvth[:]
                        )
                        mk[c] = o
                # one fat DMA per completed step; split the last step in
                # halves so the final transfer trails only chunk 3's mask
                if t == T - 1 and c == nch // 2 - 1:
                    nc.sync.dma_start(
                        out=o_d[t * P : (t + 1) * P, : (nch // 2) * fd],
                        in_=ot[:, : (nch // 2) * fd],
                    )
                elif t == T - 1 and c == nch - 1:
                    nc.sync.dma_start(
                        out=o_d[t * P : (t + 1) * P, (nch // 2) * fd :],
                        in_=ot[:, (nch // 2) * fd :],
                    )
                elif c == nch - 1:
                    nc.sync.dma_start(
                        out=o_d[t * P : (t + 1) * P, :], in_=ot[:]
                    )
    nc.compile()
    return nc


def _build_pk(spike: str = "act", xbufs: int = 12, obufs: int = 6, fd: int = FD):
    """Bit-packed output variant: one u8 byte per spatial element holding all
    T=8 spikes (bit t = o_t), cutting output HBM traffic 32x vs f32.

    Per time step t, per [P, fd] chunk c (engine assignment in parens):
      decay   u_t = TAU*u'_{t-1} + x_t        stt, in place on x tile  (Pool)
      spike   o_t = (u_t > VTH) as u8 {0,1}   (Act: Sign(u-VTH) -> u8, the
                                               -1 saturating to 0; or DVE/Pool
                                               tensor_scalar is_gt)
      pack    acc += o_t << t                 stt, acc is the u8 out tile (DVE)
      reset   u'_t = 0 where o_t              copy_predicated, mask=o_t (DVE)
    """
    import concourse.bacc as bacc
    import concourse.mybir as mybir
    import concourse.tile as tile

    nch = SPAT // (P * fd)
    nc = bacc.Bacc(
        "TRN2", target_bir_lowering=False, debug=False, num_devices=NCORES
    )
    f32 = mybir.dt.float32
    u8 = mybir.dt.uint8
    mult = mybir.AluOpType.mult
    add = mybir.AluOpType.add
    is_gt = mybir.AluOpType.is_gt
    is_le = mybir.AluOpType.is_le
    sign_f = mybir.ActivationFunctionType.Sign

    x_d = nc.dram_tensor("x", [T * nch * P, fd], f32, kind="ExternalInput").ap()
    o_d = nc.dram_tensor("o", [nch * P, fd], u8, kind="ExternalOutput").ap()

    with tile.TileContext(nc) as tc:
        with (
            tc.tile_pool(name="xp", bufs=xbufs) as xp,
            tc.tile_pool(name="op", bufs=obufs) as op_,
            tc.tile_pool(name="cp", bufs=1) as cp,
        ):
            zero = None
            nvth = None
            if spike != "ts":
                zero = cp.tile([P, fd], f32, tag="zero")
                nc.gpsimd.memset(zero[:], 0.0)
                nvth = cp.tile([P, 1], f32, tag="nvth")
                nc.gpsimd.memset(nvth[:], -VTH)
            # f32 accumulator (Pool can't do u8+u8 adds); u8 out tile is
            # written once by the final t=7 pack op.
            acc = [
                cp.tile([P, fd], f32, tag=f"acc{c}", name=f"acc{c}")
                for c in range(nch)
            ]
            out8 = [
                cp.tile([P, fd], u8, tag=f"out{c}", name=f"out{c}")
                for c in range(nch)
            ]
            st = [None] * nch
            for t in range(T):
                for c in range(nch):
                    r0 = (t * nch + c) * P
                    xt = xp.tile([P, fd], f32)
                    nc.sync.dma_start(out=xt[:], in_=x_d[r0 : r0 + P, :])
                    if t > 0:
                        # u_t = TAU*u' + x_t  (in place on the x tile).
                        # stt is DVE-only on v3 (Pool rejects TensorScalarPtr
                        # in the stt form).
                        nc.vector.scalar_tensor_tensor(
                            out=xt[:], in0=st[c][:], scalar=TAU, in1=xt[:],
                            op0=mult, op1=add,
                        )
                    st[c] = xt
                    if spike == "ts":
                        # mask-free: weighted spike + gated state, all-DVE
                        ws = op_.tile([P, fd], f32, name="wsf")
                        nc.vector.tensor_scalar(
                            ws[:], st[c][:], VTH, float(1 << t), is_gt, mult
                        )
                        if t == 0:
                            nc.vector.tensor_scalar(
                                acc[c][:], ws[:], 1.0, None, mult
                            )
                        else:
                            dst = out8[c] if t == T - 1 else acc[c]
                            nc.vector.tensor_tensor(
                                out=dst[:], in0=ws[:], in1=acc[c][:], op=add
                            )
                        if t < T - 1:
                            # u'' = (u <= VTH) * u   (kills spiked state)
                            nc.vector.scalar_tensor_tensor(
                                out=st[c][:], in0=st[c][:], scalar=VTH,
                                in1=st[c][:], op0=is_le, op1=mult,
                            )
                    else:
                        o = op_.tile([P, fd], u8)
                        if spike == "act":
                            # o = sign(u - VTH) -> u8: -1 saturates to 0
                            nc.scalar.activation(
                                o[:], st[c][:], sign_f, bias=nvth[:]
                            )
                        else:
                            eng = nc.vector if (t + c) % 2 else nc.gpsimd
                            eng.tensor_scalar(o[:], st[c][:], VTH, None, is_gt)
                        # pack: acc (f32) += o << t.  DVE already carries
                        # decay+reset (57us floor), so t<=5 pack goes to the
                        # otherwise-idle Pool engine (u8 ts, then the legal
                        # mixed u8+f32 tt add); t=6,7 are single DVE stt ops,
                        # t=7 writing the final u8 byte.
                        if t == 0:
                            nc.gpsimd.tensor_scalar(
                                acc[c][:], o[:], 1.0, None, mult
                            )
                        elif t <= 5:
                            ws = op_.tile([P, fd], u8, name="ws")
                            nc.gpsimd.tensor_scalar(
                                ws[:], o[:], float(1 << t), None, mult
                            )
                            nc.gpsimd.tensor_tensor(
                                out=acc[c][:], in0=ws[:], in1=acc[c][:],
                                op=add,
                            )
                        else:
                            dst = out8[c] if t == T - 1 else acc[c]
                            nc.vector.scalar_tensor_tensor(
                                out=dst[:], in0=o[:], scalar=float(1 << t),
                                in1=acc[c][:], op0=mult, op1=add,
                            )
                        if t < T - 1:
                            nc.vector.copy_predicated(
                                out=st[c][:], mask=o[:], data=zero[:]
                            )
                    if t == T - 1:
                        nc.sync.dma_start(
                            out=o_d[c * P : (c + 1) * P, :], in_=out8[c][:]
                        )
    nc.compile()
    return nc


def _build(
    reps: int = 1,
    mode: str = "full",
    bufs=(10, 10),
    fd=FD,
    odt: str = "f32",
    ger: str = "v",
    mer: str = "v",
    spike: str = "dve",
    pack: bool = False,
    ib: int = 2,
):
    import contextlib

    import concourse.bacc as bacc
    import concourse.mybir as mybir
    import concourse.tile as tile

    nch = SPAT // (P * fd)
    nc = bacc.Bacc(
        "TRN2",
        target_bir_lowering=False,
        debug=False,
        num_devices=NCORES,
    )
    f32 = mybir.dt.float32
    odtype = f32 if odt == "f32" else mybir.dt.int8
    if pack:
        # in rows (t, cg, p) cols (half, j); out rows (t, p) cols (c, j)
        x_d = nc.dram_tensor(
            "x", [T * (nch // ib) * P, ib * fd], f32, kind="ExternalInput"
        ).ap()
        o_d = nc.dram_tensor(
            "o", [T * P, nch * fd], mybir.dt.int8, kind="ExternalOutput"
        ).ap()
    else:
        x_d = nc.dram_tensor(
            "x", [T * nch * P, fd], f32, kind="ExternalInput"
        ).ap()
        o_d = nc.dram_tensor(
            "o", [T * nch * P, fd], odtype, kind="ExternalOutput"
        ).ap()

    with tile.TileContext(nc) as tc:
        with (
            tc.tile_pool(name="xp", bufs=bufs[0]) as xp,
            tc.tile_pool(name="op", bufs=bufs[1]) as op_,
            tc.tile_pool(name="wp", bufs=6) as wp,
            tc.tile_pool(name="cp", bufs=1) as cp,
        ):
            rep_ctx = (
                tc.For_i(0, reps, 1) if reps > 1 else contextlib.nullcontext()
            )
            with rep_ctx:
                if pack:
                    _emit_packed(nc, xp, op_, cp, x_d, o_d, mybir, mode,
                                 fd, nch, ib)
                else:
                    _emit(nc, xp, op_, wp, cp, x_d, o_d, mybir, mode, fd,
                          nch, odt, ger, mer, spike)
    nc.compile()
    return nc


def _emit_packed(nc, xp, op_, cp, x_d, o_d, mybir, mode, fd, nch, ib):
    """act1-spike i8-out variant with batched DMAs.

    Input tiles span `ib` chunks (one contiguous DMA each); output tiles
    span all `nch` chunks of a step (one contiguous DMA per step).
    """
    f32 = mybir.dt.float32
    i8 = mybir.dt.int8
    mult = mybir.AluOpType.mult
    add = mybir.AluOpType.add
    relu_f = mybir.ActivationFunctionType.Relu
    dma, compute = mode in ("full", "dma"), mode in ("full", "compute")
    ng = nch // ib

    zero = cp.tile([P, fd], f32, tag="zero")
    nc.gpsimd.memset(zero[:], 0.0)
    nvthbig = cp.tile([P, 1], f32, tag="nvthbig")
    nc.gpsimd.memset(nvthbig[:], -VTH * 1e9)

    u = [None] * nch       # AP slice holding u_t per chunk
    o_prev = [None] * nch  # AP slice of o_{t-1} per chunk
    for t in range(T):
        xts = []
        for g in range(ng):
            xt = xp.tile([P, ib * fd], f32)
            if dma:
                r0 = (t * ng + g) * P
                nc.sync.dma_start(out=xt[:], in_=x_d[r0 : r0 + P, :])
            elif t == 0:
                nc.gpsimd.memset(xt[:], 0.25)
            xts.append(xt)
        ot = op_.tile([P, nch * fd], i8)
        for c in range(nch):
            g, h = c // ib, c % ib
            xs = xts[g][:, h * fd : (h + 1) * fd]
            if compute:
                if t > 0:
                    # reset where previous step spiked
                    nc.vector.copy_predicated(
                        out=u[c], mask=o_prev[c], data=zero[:]
                    )
                    # u_t = TAU*u_masked + x_t  (in place on x slice)
                    nc.vector.scalar_tensor_tensor(
                        out=xs, in0=u[c], scalar=TAU, in1=xs,
                        op0=mult, op1=add,
                    )
                u[c] = xs
                # o8 = sat_i8(relu(1e9*u - 1e9*VTH)): nonzero iff spike
                nc.scalar.activation(
                    ot[:, c * fd : (c + 1) * fd], u[c], relu_f,
                    bias=nvthbig[:], scale=1e9,
                )
                o_prev[c] = ot[:, c * fd : (c + 1) * fd]
        if not compute:
            nc.gpsimd.memset(ot[:, :1], 1)
        if dma:
            nc.sync.dma_start(out=o_d[t * P : (t + 1) * P, :], in_=ot[:])


def _emit(nc, xp, op_, wp, cp, x_d, o_d, mybir, mode, fd, nch, odt, ger, mer,
          spike="dve"):
    f32 = mybir.dt.float32
    mult = mybir.AluOpType.mult
    add = mybir.AluOpType.add
    is_gt = mybir.AluOpType.is_gt
    copy_f = mybir.ActivationFunctionType.Copy
    dma, compute = mode in ("full", "dma"), mode in ("full", "compute")
    odtype = f32 if odt == "f32" else mybir.dt.int8
    geng = nc.vector if ger == "v" else nc.gpsimd
    meng = nc.vector if mer == "v" else nc.gpsimd

    i8 = mybir.dt.int8
    relu_f = mybir.ActivationFunctionType.Relu
    sign_f = mybir.ActivationFunctionType.Sign

    o8c = None
    if mode == "dma" and odt == "i8":
        o8c = cp.tile([P, fd], i8, tag="o8c")
        nc.gpsimd.memset(o8c[:], 1)
    if spike in ("act", "act1"):
        assert odt == "i8"
        zero = cp.tile([P, fd], f32, tag="zero")
        nc.gpsimd.memset(zero[:], 0.0)
        nvth = cp.tile([P, 1], f32, tag="nvth")
        nc.gpsimd.memset(nvth[:], -VTH)
        nvthbig = cp.tile([P, 1], f32, tag="nvthbig")
        nc.gpsimd.memset(nvthbig[:], -VTH * 1e9)

    u = [None] * nch       # tile holding u_t per chunk
    o_prev = [None] * nch  # tile holding o_{t-1} per chunk
    for t in range(T):
        for c in range(nch):
            r0 = (t * nch + c) * P
            xt = xp.tile([P, fd], f32)
            if dma:
                nc.sync.dma_start(out=xt[:], in_=x_d[r0 : r0 + P, :])
            elif t == 0:
                nc.gpsimd.memset(xt[:], 0.25)
            if compute and spike in ("act", "act1"):
                if t > 0:
                    o = o_prev[c]
                    # reset where previous step spiked
                    nc.vector.copy_predicated(
                        out=u[c][:], mask=o[:], data=zero[:]
                    )
                    # u_t = TAU*u_masked + x_t  (in place on x tile)
                    nc.vector.scalar_tensor_tensor(
                        out=xt[:], in0=u[c][:], scalar=TAU, in1=xt[:],
                        op0=mult, op1=add,
                    )
                u[c] = xt
                ot = op_.tile([P, fd], i8)
                if spike == "act1":
                    # o8 = sat_i8(relu(1e9*u - 1e9*VTH)): nonzero iff spike.
                    # int8 conversion saturates at 127 (verified on HW), and
                    # |u-VTH| >= 1 ulp(1.5) so the *1e9 rounding never
                    # crosses zero.
                    nc.scalar.activation(
                        ot[:], u[c][:], relu_f, bias=nvthbig[:], scale=1e9
                    )
                else:
                    # spike on ScalarE: sg = sign(u - VTH); o = relu(sg)
                    sg = wp.tile([P, fd], f32, tag="sg")
                    nc.scalar.activation(sg[:], u[c][:], sign_f, bias=nvth[:])
                    nc.scalar.activation(ot[:], sg[:], relu_f)
                o_prev[c] = ot
            elif compute:
                if t == 0:
                    u[c] = xt
                else:
                    o = o_prev[c]
                    if odt == "f32":
                        # w <- TAU - TAU*o  (in place over o after its store)
                        w = o
                        nc.scalar.activation(
                            w[:], o[:], copy_f, bias=TAU, scale=-TAU
                        )
                    else:
                        w = wp.tile([P, fd], f32)
                        nc.scalar.activation(
                            w[:], o[:], copy_f, bias=TAU, scale=-TAU
                        )
                    # u_masked = u_{t-1} * w   (in place)
                    meng.tensor_tensor(
                        out=u[c][:], in0=u[c][:], in1=w[:], op=mult
                    )
                    # u_t = u_masked + x_t    (in place on x tile)
                    nc.vector.tensor_tensor(
                        out=xt[:], in0=u[c][:], in1=xt[:], op=add
                    )
                    u[c] = xt
                ot = op_.tile([P, fd], odtype)
                geng.tensor_scalar(ot[:], u[c][:], VTH, None, is_gt)
                o_prev[c] = ot
            else:
                ot = o8c if o8c is not None else xt
            if dma:
                nc.sync.dma_start(out=o_d[r0 : r0 + P, :], in_=ot[:])


def _mode():
    import os

    return os.environ.get("LIF_MODE", "pe")


def _get_compiled():
    global _compiled
    if _compiled is None:
        import os

        mode = _mode()
        if mode == "pe":
            _compiled = _build_pe(
                fd=int(os.environ.get("LIF_FD", "1024")),
                kadd=int(os.environ.get("LIF_KADD", "6")),
                meng=os.environ.get("LIF_MENG", "pool"),
                xbufs=int(os.environ.get("LIF_XBUFS", "16")),
                pbufs=int(os.environ.get("LIF_PBUFS", "4")),
            )
        elif mode == "pk":
            _compiled = _build_pk(
                spike=os.environ.get("LIF_SPIKE", "act"),
                xbufs=int(os.environ.get("LIF_XBUFS", "12")),
                obufs=int(os.environ.get("LIF_OBUFS", "6")),
            )
        elif mode == "act1":
            _compiled = _build(spike="act1", odt="i8")
        elif mode == "h":
            _compiled = _build_h(
                jm7=int(os.environ.get("LIF_JM7", "4")),
                xbufs=int(os.environ.get("LIF_XBUFS", "12")),
                obufs=int(os.environ.get("LIF_OBUFS", "8")),
                mulf=os.environ.get("LIF_MULF", "0") == "1",
            )
        else:
            _compiled = _build()
    return _compiled


def _shard_pe(x: np.ndarray, i: int, fd: int) -> np.ndarray:
    """Core i's shard for pe mode: chunk-major [(c,t,p), fd] rows, with
    x_t pre-scaled by 10^t (v-domain)."""
    nch = SPAT // (P * fd)
    xs = x[i * BS : (i + 1) * BS].reshape(SPAT, T)
    xv = xs * np.asarray(VSCALE, dtype=np.float32)[None, :]
    xv = xv.reshape(nch, P, fd, T).transpose(0, 3, 1, 2)  # [c, t, P, fd]
    return np.ascontiguousarray(xv).reshape(nch * T * P, fd)


def _w_pe() -> np.ndarray:
    import ml_dtypes

    eye = np.eye(P, dtype=np.float32)
    w = np.concatenate([eye * float(1 << t) for t in range(T)], axis=0)
    return w.astype(ml_dtypes.bfloat16)


def _shard_tmajor(x: np.ndarray, i: int) -> np.ndarray:
    """Core i's shard, time-major: [T*NCH*P, FD], row-major over (t, spatial)."""
    xs = x[i * BS : (i + 1) * BS]                   # [BS,C,H,W,T]
    xt = np.moveaxis(xs.reshape(SPAT, T), -1, 0)    # [T, SPAT]
    return np.ascontiguousarray(xt).reshape(ROWS, FD)


def kernel(x: np.ndarray, _trace: bool = False):
    nc = _get_compiled()
    from concourse.bass_utils import run_bass_kernel_spmd

    x = np.asarray(x, dtype=np.float32)
    if _mode() == "pe":
        import os

        fd = int(os.environ.get("LIF_FD", "1024"))
        w = _w_pe()
        in_maps = [
            {"x": _shard_pe(x, i, fd), "w": w} for i in range(NCORES)
        ]
    else:
        in_maps = [{"x": _shard_tmajor(x, i)} for i in range(NCORES)]
    res = run_bass_kernel_spmd(
        nc, in_maps, core_ids=list(range(NCORES)), trace=_trace
    )
    invert = _mode() == "pe"                        # pe packs the keep-mask
    outs = []
    for r in res.results:
        ot = r["o"]
        if ot.size == SPAT:                         # bit-packed u8: bit t = o_t
            if invert:
                ot = np.invert(ot)
            bits = np.unpackbits(
                ot.reshape(-1, 1), axis=1, bitorder="little"
            )[:, :T]
            outs.append(bits.reshape(BS, C, H, W, T).astype(np.float32))
            continue
        if _mode() == "h":                          # u8 masks, mixed polarity
            import os

            jm7 = int(os.environ.get("LIF_JM7", "4"))
            mulf = os.environ.get("LIF_MULF", "0") == "1"
            nch = SPAT // (P * FD)
            # rows (t,p), cols (c,j) -> [T, nch, P, FD]
            bits = (ot != 0).reshape(T, P, nch, FD).transpose(0, 2, 1, 3)
            if not mulf:                            # keep-mask blocks invert
                for t in range(T - 1):
                    for c in range(nch):
                        if _h_ismul(t, c, nch, jm7):
                            bits[t, c] = ~bits[t, c]
            ot = bits.reshape(T, SPAT).astype(np.float32)
            outs.append(np.moveaxis(ot, 0, -1).reshape(BS, C, H, W, T))
            continue
        if ot.dtype != np.float32:                  # int8 spikes -> f32
            ot = (ot != 0).astype(np.float32)
        ot = ot.reshape(T, SPAT)                    # time-major back to T-last
        outs.append(np.moveaxis(ot, 0, -1).reshape(BS, C, H, W, T))
    out = np.ascontiguousarray(np.concatenate(outs, axis=0))
    return (out, res) if _trace else out



# revision 34
# speedup vs baseline: 5.9704x; 1.0059x over previous
"""LIF spike (leaky integrate-and-fire) forward kernel for Trainium2.

Recurrence over the time axis T=8 of x[64,128,32,32,8] (fp32):
    u_t = TAU * u_{t-1} * (1 - o_{t-1}) + x_t
    o_t = (u_t > VTH)
Data-parallel over the batch dim: 8 NeuronCores x 8 batches each.

Shipped design (mode "h", jm7=0): time-major [128, 2048] f32 tiles, per
step t and chunk c:
    reset  u' = 0 where o_{t-1}      DVE copy_predicated, u8 mask  2.42us
    state  u_t = TAU*u' + x_t        DVE scalar_tensor_tensor      2.28us
    spike  o_t = sign(u_t-VTH)->u8   Act (saturates -1 to 0)       2.00us
The Act mask tile is also the output byte (u8 {0,1}); output rides one
fat [128, nch*2048] staging tile per step (8KB DMA rows).  Bit-exact vs
the jax reference.

Measured (HW, ntff profile, core 0): exec ~153us = ~15us DMA startup +
~129us of gap-free DVE + ~8us tail; staged predecessor measured 213us
under the same methodology.  Engine notes from this hardware: DVE ts
1-src 1.21us/tile (2x mode), DVE 2-src f32 ops ~2.3us, Act always
2.0us; GpSimd(Pool) is unusable (tensor_scalar ~29us/tile, tensor_tensor
~4.5-6us, and ANY concurrent Pool op inflates DVE ops 30-60% via SBUF
contention); mixed-dtype (f32 x u8/bf16) 2-src ops are ~6us on either
engine; stt/copy_predicated are DVE-only (Pool rejects the stt opcode,
copy_predicated needs an integer mask dtype).  Output traffic is i8
(42MB/core total vs 67MB for f32 out); bit-packing to 1MB was designed
(PE identity-matmul accumulate) but doesn't pay because DVE, not DMA,
is the binding constraint at 131us busy.
"""

import sys

for _p in ("/opt/trn_rl_repo",):
    if _p not in sys.path:
        sys.path.insert(0, _p)

import numpy as np

TAU = 0.1
VTH = 1.5

B, C, H, W, T = 64, 128, 32, 32, 8
NCORES = 8
BS = B // NCORES                      # batches per core
SPAT = BS * C * H * W                 # spatial elems per core per step: 1,048,576
P = 128                               # partitions
FD = 2048                             # free dim per tile
NCH = SPAT // (P * FD)                # spatial chunks per step: 4
ROWS = T * NCH * P                    # dram rows (t-major): 4096
ELEMS = SPAT * T

_compiled = None

# v-domain scaling: v_t = 10^t * u_t kills the TAU multiply (host pre-scales
# x_t by 10^t); thresholds 1.5*10^t are all exact in f32.
VSCALE = [float(10.0**t) for t in range(T)]
VTH_T = [float(1.5 * 10.0**t) for t in range(T)]


def _build_pe(fd: int = 1024, kadd: int = 6, meng: str = "pool",
              xbufs: int = 16, mbufs: int = 8, pbufs: int = 4):
    """v-domain LIF with PE-packed output bytes.

    Recurrence per chunk c (sequential in t):
        v_t = v_{t-1} * m_{t-1} + xs_t      xs_t = 10^t * x_t (host-scaled)
        m_t = (v_t <= 1.5*10^t)             keep-mask, bf16 {0,1}  (DVE ts)
    Packing on the otherwise-idle PE: psum += (2^t I) @ m_t over the 8 steps
    gives byte = sum_t m_t 2^t (exact: bf16 holds {0,1} and 2^t; PSUM is
    f32).  Act copies PSUM -> SBUF u8; host inverts bits (o = NOT m).
    The reset multiply runs on Pool (pure-ish tt), adds split DVE/Pool via
    `kadd` (# adds per chunk on DVE).
    """
    import concourse.bacc as bacc
    import concourse.mybir as mybir
    import concourse.tile as tile

    nch = SPAT // (P * fd)
    nc = bacc.Bacc(
        "TRN2", target_bir_lowering=False, debug=False, num_devices=NCORES
    )
    f32 = mybir.dt.float32
    bf16 = mybir.dt.bfloat16
    u8 = mybir.dt.uint8
    mult = mybir.AluOpType.mult
    add = mybir.AluOpType.add
    is_le = mybir.AluOpType.is_le
    copy_f = mybir.ActivationFunctionType.Copy

    x_d = nc.dram_tensor(
        "x", [nch * T * P, fd], f32, kind="ExternalInput"
    ).ap()
    w_d = nc.dram_tensor("w", [T * P, P], bf16, kind="ExternalInput").ap()
    o_d = nc.dram_tensor("o", [nch * P, fd], u8, kind="ExternalOutput").ap()

    with tile.TileContext(nc) as tc:
        with (
            tc.tile_pool(name="xp", bufs=xbufs) as xp,
            tc.tile_pool(name="mp", bufs=mbufs) as mp,
            tc.tile_pool(name="op", bufs=2) as op_,
            tc.tile_pool(name="wp", bufs=1) as wp,
            tc.psum_pool(name="pp", bufs=pbufs) as pp,
        ):
            wts = []
            for t in range(T):
                wt = wp.tile([P, P], bf16, tag=f"w{t}", name=f"w{t}")
                nc.sync.dma_start(out=wt[:], in_=w_d[t * P : (t + 1) * P, :])
                wts.append(wt)
            for c in range(nch):
                ps = pp.tile([P, fd], f32, name="ps")
                st = None
                mprev = None
                for t in range(T):
                    r0 = (c * T + t) * P
                    xt = xp.tile([P, fd], f32)
                    nc.sync.dma_start(out=xt[:], in_=x_d[r0 : r0 + P, :])
                    if t > 0:
                        # um = v_{t-1} * m_{t-1}  (in place on state tile)
                        me = nc.gpsimd if meng == "pool" else nc.vector
                        me.tensor_tensor(
                            out=st[:], in0=st[:], in1=mprev[:], op=mult
                        )
                        # v_t = um + xs_t  (in place on the x tile)
                        ae = nc.vector if t <= kadd else nc.gpsimd
                        ae.tensor_tensor(
                            out=xt[:], in0=st[:], in1=xt[:], op=add
                        )
                    st = xt
                    m = mp.tile([P, fd], bf16)
                    nc.vector.tensor_scalar(
                        m[:], st[:], VTH_T[t], None, is_le
                    )
                    # PSUM bank limit: <=512 f32 out columns per matmul
                    for h in range(fd // 512):
                        sl = slice(h * 512, (h + 1) * 512)
                        nc.tensor.matmul(
                            ps[:, sl], wts[t][:], m[:, sl],
                            start=(t == 0), stop=(t == T - 1),
                        )
                    mprev = m
                ot = op_.tile([P, fd], u8)
                nc.scalar.activation(ot[:], ps[:], copy_f)
                nc.sync.dma_start(
                    out=o_d[c * P : (c + 1) * P, :], in_=ot[:]
                )
    nc.compile()
    return nc


def _h_ismul(t: int, c: int, nch: int, jm7: int) -> bool:
    """Static per-quantum choice: True = reset via Pool multiply (Act emits
    the keep-mask), False = reset via DVE copy_predicated (Act emits the
    spike-mask).  t is the step whose mask this is (0..T-2)."""
    return (t * nch + c) % 7 < jm7


def _build_h(fd: int = FD, jm7: int = 0, xbufs: int = 16, obufs: int = 3,
             mulf: bool = False):
    """i8-out hybrid: per step the Act engine emits one u8 {0,1} mask tile
    (spike- or keep-oriented), which is both the reset selector and the DMA'd
    output byte (host re-inverts keep-oriented blocks).

    Per step t>0, chunk c (28 update quanta):
      cp path:   u' = 0 where o_prev      DVE copy_predicated (2.4us/q)
      mul path:  u' = u * m_prev          Pool mixed u8*f32 tt (6.0us/q)
      then       u_t = TAU*u' + x_t       DVE stt (2.3us/q)
      mask       Act Sign(+-(u-VTH))      2.0us/q
    jm7/7 of quanta take the mul path, balancing DVE ~93us / Pool ~96us
    under the 42MB DMA wall (~118us @ 356GB/s measured).
    """
    import concourse.bacc as bacc
    import concourse.mybir as mybir
    import concourse.tile as tile

    nch = SPAT // (P * fd)
    nc = bacc.Bacc(
        "TRN2", target_bir_lowering=False, debug=False, num_devices=NCORES
    )
    f32 = mybir.dt.float32
    u8 = mybir.dt.uint8
    mult = mybir.AluOpType.mult
    add = mybir.AluOpType.add
    is_le = mybir.AluOpType.is_le
    sign_f = mybir.ActivationFunctionType.Sign

    x_d = nc.dram_tensor("x", [T * nch * P, fd], f32, kind="ExternalInput").ap()
    # one fat row block per step: 8KB rows for efficient output DMA
    o_d = nc.dram_tensor("o", [T * P, nch * fd], u8, kind="ExternalOutput").ap()

    with tile.TileContext(nc) as tc:
        with (
            tc.tile_pool(name="xp", bufs=xbufs) as xp,
            tc.tile_pool(name="op", bufs=obufs) as op_,
            tc.tile_pool(name="mp", bufs=4) as mp,
            tc.tile_pool(name="cp", bufs=1) as cp,
        ):
            zero = cp.tile([P, fd], f32, tag="zero")
            nc.gpsimd.memset(zero[:], 0.0)
            nvth = cp.tile([P, 1], f32, tag="nvth")
            nc.gpsimd.memset(nvth[:], -VTH)
            pvth = cp.tile([P, 1], f32, tag="pvth")
            nc.gpsimd.memset(pvth[:], VTH)

            st = [None] * nch    # state tile per chunk
            mk = [None] * nch    # mask AP (slice of the staging tile)
            # quantum order: steps 0/1 interleaved per chunk so the first
            # stt only waits on two 1MiB loads, then t-major
            order = []
            for c in range(nch):
                order += [(0, c), (1, c)]
            for t in range(2, T):
                order += [(t, c) for c in range(nch)]
            ots = {}
            for t, c in order:
                if c == 0:
                    ots[t] = op_.tile([P, nch * fd], u8, name="ot")
                ot = ots[t]
                if True:
                    r0 = (t * nch + c) * P
                    xt = xp.tile([P, fd], f32)
                    nc.sync.dma_start(out=xt[:], in_=x_d[r0 : r0 + P, :])
                    if t > 0:
                        if _h_ismul(t - 1, c, nch, jm7):
                            if mulf:
                                # pure-f32: u' = u * m_f32  (Pool, no u8)
                                nc.gpsimd.tensor_tensor(
                                    out=st[c][:], in0=st[c][:], in1=mk[c],
                                    op=mult,
                                )
                            else:
                                # keep-mask: u' = u * m  (mixed u8*f32, Pool)
                                nc.gpsimd.tensor_tensor(
                                    out=st[c][:], in0=st[c][:], in1=mk[c],
                                    op=mult,
                                )
                        else:
                            # spike-mask: zero u where spiked (DVE)
                            nc.vector.copy_predicated(
                                out=st[c][:], mask=mk[c], data=zero[:]
                            )
                        # u_t = TAU*u' + x_t  (in place on x tile, DVE)
                        nc.vector.scalar_tensor_tensor(
                            out=xt[:], in0=st[c][:], scalar=TAU, in1=xt[:],
                            op0=mult, op1=add,
                        )
                    st[c] = xt
                    o = ot[:, c * fd : (c + 1) * fd]
                    if mulf:
                        # output always spike-oriented
                        nc.scalar.activation(
                            o, st[c][:], sign_f, bias=nvth[:]
                        )
                        if t < T - 1 and _h_ismul(t, c, nch, jm7):
                            # f32 keep-mask for the Pool multiply (DVE ts 2x)
                            mf = mp.tile([P, fd], f32, name="mf")
                            nc.vector.tensor_scalar(
                                mf[:], st[c][:], VTH, None, is_le
                            )
                            mk[c] = mf[:]
                        else:
                            mk[c] = o
                    elif t < T - 1 and _h_ismul(t, c, nch, jm7):
                        # m = sign(VTH - u) -> u8 {0,1}: keep-mask
                        nc.scalar.activation(
                            o, st[c][:], sign_f, bias=pvth[:], scale=-1.0
                        )
                        mk[c] = o
                    else:
                        # o = sign(u - VTH) -> u8 {0,1}: spike-mask
                        nc.scalar.activation(
                            o, st[c][:], sign_f, bias=nvth[:]
                        )
                        mk[c] = o
                # one fat DMA per completed step; split the last step in
                # halves so the final transfer trails only chunk 3's mask
                if t == T - 1 and c == nch // 2 - 1:
                    nc.sync.dma_start(
                        out=o_d[t * P : (t + 1) * P, : (nch // 2) * fd],
                        in_=ot[:, : (nch // 2) * fd],
                    )
                elif t == T - 1 and c == nch - 1:
                    nc.sync.dma_start(
                        out=o_d[t * P : (t + 1) * P, (nch // 2) * fd :],
                        in_=ot[:, (nch // 2) * fd :],
                    )
                elif c == nch - 1:
                    nc.sync.dma_start(
                        out=o_d[t * P : (t + 1) * P, :], in_=ot[:]
                    )
    nc.compile()
    return nc


def _build_pk(spike: str = "act", xbufs: int = 12, obufs: int = 6, fd: int = FD):
    """Bit-packed output variant: one u8 byte per spatial element holding all
    T=8 spikes (bit t = o_t), cutting output HBM traffic 32x vs f32.

    Per time step t, per [P, fd] chunk c (engine assignment in parens):
      decay   u_t = TAU*u'_{t-1} + x_t        stt, in place on x tile  (Pool)
      spike   o_t = (u_t > VTH) as u8 {0,1}   (Act: Sign(u-VTH) -> u8, the
                                               -1 saturating to 0; or DVE/Pool
                                               tensor_scalar is_gt)
      pack    acc += o_t << t                 stt, acc is the u8 out tile (DVE)
      reset   u'_t = 0 where o_t              copy_predicated, mask=o_t (DVE)
    """
    import concourse.bacc as bacc
    import concourse.mybir as mybir
    import concourse.tile as tile

    nch = SPAT // (P * fd)
    nc = bacc.Bacc(
        "TRN2", target_bir_lowering=False, debug=False, num_devices=NCORES
    )
    f32 = mybir.dt.float32
    u8 = mybir.dt.uint8
    mult = mybir.AluOpType.mult
    add = mybir.AluOpType.add
    is_gt = mybir.AluOpType.is_gt
    is_le = mybir.AluOpType.is_le
    sign_f = mybir.ActivationFunctionType.Sign

    x_d = nc.dram_tensor("x", [T * nch * P, fd], f32, kind="ExternalInput").ap()
    o_d = nc.dram_tensor("o", [nch * P, fd], u8, kind="ExternalOutput").ap()

    with tile.TileContext(nc) as tc:
        with (
            tc.tile_pool(name="xp", bufs=xbufs) as xp,
            tc.tile_pool(name="op", bufs=obufs) as op_,
            tc.tile_pool(name="cp", bufs=1) as cp,
        ):
            zero = None
            nvth = None
            if spike != "ts":
                zero = cp.tile([P, fd], f32, tag="zero")
                nc.gpsimd.memset(zero[:], 0.0)
                nvth = cp.tile([P, 1], f32, tag="nvth")
                nc.gpsimd.memset(nvth[:], -VTH)
            # f32 accumulator (Pool can't do u8+u8 adds); u8 out tile is
            # written once by the final t=7 pack op.
            acc = [
                cp.tile([P, fd], f32, tag=f"acc{c}", name=f"acc{c}")
                for c in range(nch)
            ]
            out8 = [
                cp.tile([P, fd], u8, tag=f"out{c}", name=f"out{c}")
                for c in range(nch)
            ]
            st = [None] * nch
            for t in range(T):
                for c in range(nch):
                    r0 = (t * nch + c) * P
                    xt = xp.tile([P, fd], f32)
                    nc.sync.dma_start(out=xt[:], in_=x_d[r0 : r0 + P, :])
                    if t > 0:
                        # u_t = TAU*u' + x_t  (in place on the x tile).
                        # stt is DVE-only on v3 (Pool rejects TensorScalarPtr
                        # in the stt form).
                        nc.vector.scalar_tensor_tensor(
                            out=xt[:], in0=st[c][:], scalar=TAU, in1=xt[:],
                            op0=mult, op1=add,
                        )
                    st[c] = xt
                    if spike == "ts":
                        # mask-free: weighted spike + gated state, all-DVE
                        ws = op_.tile([P, fd], f32, name="wsf")
                        nc.vector.tensor_scalar(
                            ws[:], st[c][:], VTH, float(1 << t), is_gt, mult
                        )
                        if t == 0:
                            nc.vector.tensor_scalar(
                                acc[c][:], ws[:], 1.0, None, mult
                            )
                        else:
                            dst = out8[c] if t == T - 1 else acc[c]
                            nc.vector.tensor_tensor(
                                out=dst[:], in0=ws[:], in1=acc[c][:], op=add
                            )
                        if t < T - 1:
                            # u'' = (u <= VTH) * u   (kills spiked state)
                            nc.vector.scalar_tensor_tensor(
                                out=st[c][:], in0=st[c][:], scalar=VTH,
                                in1=st[c][:], op0=is_le, op1=mult,
                            )
                    else:
                        o = op_.tile([P, fd], u8)
                        if spike == "act":
                            # o = sign(u - VTH) -> u8: -1 saturates to 0
                            nc.scalar.activation(
                                o[:], st[c][:], sign_f, bias=nvth[:]
                            )
                        else:
                            eng = nc.vector if (t + c) % 2 else nc.gpsimd
                            eng.tensor_scalar(o[:], st[c][:], VTH, None, is_gt)
                        # pack: acc (f32) += o << t.  DVE already carries
                        # decay+reset (57us floor), so t<=5 pack goes to the
                        # otherwise-idle Pool engine (u8 ts, then the legal
                        # mixed u8+f32 tt add); t=6,7 are single DVE stt ops,
                        # t=7 writing the final u8 byte.
                        if t == 0:
                            nc.gpsimd.tensor_scalar(
                                acc[c][:], o[:], 1.0, None, mult
                            )
                        elif t <= 5:
                            ws = op_.tile([P, fd], u8, name="ws")
                            nc.gpsimd.tensor_scalar(
                                ws[:], o[:], float(1 << t), None, mult
                            )
                            nc.gpsimd.tensor_tensor(
                                out=acc[c][:], in0=ws[:], in1=acc[c][:],
                                op=add,
                            )
                        else:
                            dst = out8[c] if t == T - 1 else acc[c]
                            nc.vector.scalar_tensor_tensor(
                                out=dst[:], in0=o[:], scalar=float(1 << t),
                                in1=acc[c][:], op0=mult, op1=add,
                            )
                        if t < T - 1:
                            nc.vector.copy_predicated(
                                out=st[c][:], mask=o[:], data=zero[:]
                            )
                    if t == T - 1:
                        nc.sync.dma_start(
                            out=o_d[c * P : (c + 1) * P, :], in_=out8[c][:]
                        )
    nc.compile()
    return nc


def _build(
    reps: int = 1,
    mode: str = "full",
    bufs=(10, 10),
    fd=FD,
    odt: str = "f32",
    ger: str = "v",
    mer: str = "v",
    spike: str = "dve",
    pack: bool = False,
    ib: int = 2,
):
    import contextlib

    import concourse.bacc as bacc
    import concourse.mybir as mybir
    import concourse.tile as tile

    nch = SPAT // (P * fd)
    nc = bacc.Bacc(
        "TRN2",
        target_bir_lowering=False,
        debug=False,
        num_devices=NCORES,
    )
    f32 = mybir.dt.float32
    odtype = f32 if odt == "f32" else mybir.dt.int8
    if pack:
        # in rows (t, cg, p) cols (half, j); out rows (t, p) cols (c, j)
        x_d = nc.dram_tensor(
            "x", [T * (nch // ib) * P, ib * fd], f32, kind="ExternalInput"
        ).ap()
        o_d = nc.dram_tensor(
            "o", [T * P, nch * fd], mybir.dt.int8, kind="ExternalOutput"
        ).ap()
    else:
        x_d = nc.dram_tensor(
            "x", [T * nch * P, fd], f32, kind="ExternalInput"
        ).ap()
        o_d = nc.dram_tensor(
            "o", [T * nch * P, fd], odtype, kind="ExternalOutput"
        ).ap()

    with tile.TileContext(nc) as tc:
        with (
            tc.tile_pool(name="xp", bufs=bufs[0]) as xp,
            tc.tile_pool(name="op", bufs=bufs[1]) as op_,
            tc.tile_pool(name="wp", bufs=6) as wp,
            tc.tile_pool(name="cp", bufs=1) as cp,
        ):
            rep_ctx = (
                tc.For_i(0, reps, 1) if reps > 1 else contextlib.nullcontext()
            )
            with rep_ctx:
                if pack:
                    _emit_packed(nc, xp, op_, cp, x_d, o_d, mybir, mode,
                                 fd, nch, ib)
                else:
                    _emit(nc, xp, op_, wp, cp, x_d, o_d, mybir, mode, fd,
                          nch, odt, ger, mer, spike)
    nc.compile()
    return nc


def _emit_packed(nc, xp, op_, cp, x_d, o_d, mybir, mode, fd, nch, ib):
    """act1-spike i8-out variant with batched DMAs.

    Input tiles span `ib` chunks (one contiguous DMA each); output tiles
    span all `nch` chunks of a step (one contiguous DMA per step).
    """
    f32 = mybir.dt.float32
    i8 = mybir.dt.int8
    mult = mybir.AluOpType.mult
    add = mybir.AluOpType.add
    relu_f = mybir.ActivationFunctionType.Relu
    dma, compute = mode in ("full", "dma"), mode in ("full", "compute")
    ng = nch // ib

    zero = cp.tile([P, fd], f32, tag="zero")
    nc.gpsimd.memset(zero[:], 0.0)
    nvthbig = cp.tile([P, 1], f32, tag="nvthbig")
    nc.gpsimd.memset(nvthbig[:], -VTH * 1e9)

    u = [None] * nch       # AP slice holding u_t per chunk
    o_prev = [None] * nch  # AP slice of o_{t-1} per chunk
    for t in range(T):
        xts = []
        for g in range(ng):
            xt = xp.tile([P, ib * fd], f32)
            if dma:
                r0 = (t * ng + g) * P
                nc.sync.dma_start(out=xt[:], in_=x_d[r0 : r0 + P, :])
            elif t == 0:
                nc.gpsimd.memset(xt[:], 0.25)
            xts.append(xt)
        ot = op_.tile([P, nch * fd], i8)
        for c in range(nch):
            g, h = c // ib, c % ib
            xs = xts[g][:, h * fd : (h + 1) * fd]
            if compute:
                if t > 0:
                    # reset where previous step spiked
                    nc.vector.copy_predicated(
                        out=u[c], mask=o_prev[c], data=zero[:]
                    )
                    # u_t = TAU*u_masked + x_t  (in place on x slice)
                    nc.vector.scalar_tensor_tensor(
                        out=xs, in0=u[c], scalar=TAU, in1=xs,
                        op0=mult, op1=add,
                    )
                u[c] = xs
                # o8 = sat_i8(relu(1e9*u - 1e9*VTH)): nonzero iff spike
                nc.scalar.activation(
                    ot[:, c * fd : (c + 1) * fd], u[c], relu_f,
                    bias=nvthbig[:], scale=1e9,
                )
                o_prev[c] = ot[:, c * fd : (c + 1) * fd]
        if not compute:
            nc.gpsimd.memset(ot[:, :1], 1)
        if dma:
            nc.sync.dma_start(out=o_d[t * P : (t + 1) * P, :], in_=ot[:])


def _emit(nc, xp, op_, wp, cp, x_d, o_d, mybir, mode, fd, nch, odt, ger, mer,
          spike="dve"):
    f32 = mybir.dt.float32
    mult = mybir.AluOpType.mult
    add = mybir.AluOpType.add
    is_gt = mybir.AluOpType.is_gt
    copy_f = mybir.ActivationFunctionType.Copy
    dma, compute = mode in ("full", "dma"), mode in ("full", "compute")
    odtype = f32 if odt == "f32" else mybir.dt.int8
    geng = nc.vector if ger == "v" else nc.gpsimd
    meng = nc.vector if mer == "v" else nc.gpsimd

    i8 = mybir.dt.int8
    relu_f = mybir.ActivationFunctionType.Relu
    sign_f = mybir.ActivationFunctionType.Sign

    o8c = None
    if mode == "dma" and odt == "i8":
        o8c = cp.tile([P, fd], i8, tag="o8c")
        nc.gpsimd.memset(o8c[:], 1)
    if spike in ("act", "act1"):
        assert odt == "i8"
        zero = cp.tile([P, fd], f32, tag="zero")
        nc.gpsimd.memset(zero[:], 0.0)
        nvth = cp.tile([P, 1], f32, tag="nvth")
        nc.gpsimd.memset(nvth[:], -VTH)
        nvthbig = cp.tile([P, 1], f32, tag="nvthbig")
        nc.gpsimd.memset(nvthbig[:], -VTH * 1e9)

    u = [None] * nch       # tile holding u_t per chunk
    o_prev = [None] * nch  # tile holding o_{t-1} per chunk
    for t in range(T):
        for c in range(nch):
            r0 = (t * nch + c) * P
            xt = xp.tile([P, fd], f32)
            if dma:
                nc.sync.dma_start(out=xt[:], in_=x_d[r0 : r0 + P, :])
            elif t == 0:
                nc.gpsimd.memset(xt[:], 0.25)
            if compute and spike in ("act", "act1"):
                if t > 0:
                    o = o_prev[c]
                    # reset where previous step spiked
                    nc.vector.copy_predicated(
                        out=u[c][:], mask=o[:], data=zero[:]
                    )
                    # u_t = TAU*u_masked + x_t  (in place on x tile)
                    nc.vector.scalar_tensor_tensor(
                        out=xt[:], in0=u[c][:], scalar=TAU, in1=xt[:],
                        op0=mult, op1=add,
                    )
                u[c] = xt
                ot = op_.tile([P, fd], i8)
                if spike == "act1":
                    # o8 = sat_i8(relu(1e9*u - 1e9*VTH)): nonzero iff spike.
                    # int8 conversion saturates at 127 (verified on HW), and
                    # |u-VTH| >= 1 ulp(1.5) so the *1e9 rounding never
                    # crosses zero.
                    nc.scalar.activation(
                        ot[:], u[c][:], relu_f, bias=nvthbig[:], scale=1e9
                    )
                else:
                    # spike on ScalarE: sg = sign(u - VTH); o = relu(sg)
                    sg = wp.tile([P, fd], f32, tag="sg")
                    nc.scalar.activation(sg[:], u[c][:], sign_f, bias=nvth[:])
                    nc.scalar.activation(ot[:], sg[:], relu_f)
                o_prev[c] = ot
            elif compute:
                if t == 0:
                    u[c] = xt
                else:
                    o = o_prev[c]
                    if odt == "f32":
                        # w <- TAU - TAU*o  (in place over o after its store)
                        w = o
                        nc.scalar.activation(
                            w[:], o[:], copy_f, bias=TAU, scale=-TAU
                        )
                    else:
                        w = wp.tile([P, fd], f32)
                        nc.scalar.activation(
                            w[:], o[:], copy_f, bias=TAU, scale=-TAU
                        )
                    # u_masked = u_{t-1} * w   (in place)
                    meng.tensor_tensor(
                        out=u[c][:], in0=u[c][:], in1=w[:], op=mult
                    )
                    # u_t = u_masked + x_t    (in place on x tile)
                    nc.vector.tensor_tensor(
                        out=xt[:], in0=u[c][:], in1=xt[:], op=add
                    )
                    u[c] = xt
                ot = op_.tile([P, fd], odtype)
                geng.tensor_scalar(ot[:], u[c][:], VTH, None, is_gt)
                o_prev[c] = ot
            else:
                ot = o8c if o8c is not None else xt
            if dma:
                nc.sync.dma_start(out=o_d[r0 : r0 + P, :], in_=ot[:])


def _mode():
    import os

    return os.environ.get("LIF_MODE", "h")


def _get_compiled():
    global _compiled
    if _compiled is None:
        import os

        mode = _mode()
        if mode == "pe":
            _compiled = _build_pe(
                fd=int(os.environ.get("LIF_FD", "1024")),
                kadd=int(os.environ.get("LIF_KADD", "6")),
                meng=os.environ.get("LIF_MENG", "pool"),
                xbufs=int(os.environ.get("LIF_XBUFS", "16")),
                pbufs=int(os.environ.get("LIF_PBUFS", "4")),
            )
        elif mode == "pk":
            _compiled = _build_pk(
                spike=os.environ.get("LIF_SPIKE", "act"),
                xbufs=int(os.environ.get("LIF_XBUFS", "12")),
                obufs=int(os.environ.get("LIF_OBUFS", "6")),
            )
        elif mode == "act1":
            _compiled = _build(spike="act1", odt="i8")
        elif mode == "h":
            _compiled = _build_h(
                jm7=int(os.environ.get("LIF_JM7", "0")),
                xbufs=int(os.environ.get("LIF_XBUFS", "16")),
                obufs=int(os.environ.get("LIF_OBUFS", "3")),
                mulf=os.environ.get("LIF_MULF", "0") == "1",
            )
        else:
            _compiled = _build()
    return _compiled


def _shard_pe(x: np.ndarray, i: int, fd: int) -> np.ndarray:
    """Core i's shard for pe mode: chunk-major [(c,t,p), fd] rows, with
    x_t pre-scaled by 10^t (v-domain)."""
    nch = SPAT // (P * fd)
    xs = x[i * BS : (i + 1) * BS].reshape(SPAT, T)
    xv = xs * np.asarray(VSCALE, dtype=np.float32)[None, :]
    xv = xv.reshape(nch, P, fd, T).transpose(0, 3, 1, 2)  # [c, t, P, fd]
    return np.ascontiguousarray(xv).reshape(nch * T * P, fd)


def _w_pe() -> np.ndarray:
    import ml_dtypes

    eye = np.eye(P, dtype=np.float32)
    w = np.concatenate([eye * float(1 << t) for t in range(T)], axis=0)
    return w.astype(ml_dtypes.bfloat16)


def _shard_tmajor(x: np.ndarray, i: int) -> np.ndarray:
    """Core i's shard, time-major: [T*NCH*P, FD], row-major over (t, spatial)."""
    xs = x[i * BS : (i + 1) * BS]                   # [BS,C,H,W,T]
    xt = np.moveaxis(xs.reshape(SPAT, T), -1, 0)    # [T, SPAT]
    return np.ascontiguousarray(xt).reshape(ROWS, FD)


def kernel(x: np.ndarray, _trace: bool = False):
    nc = _get_compiled()
    from concourse.bass_utils import run_bass_kernel_spmd

    x = np.asarray(x, dtype=np.float32)
    if _mode() == "pe":
        import os

        fd = int(os.environ.get("LIF_FD", "1024"))
        w = _w_pe()
        in_maps = [
            {"x": _shard_pe(x, i, fd), "w": w} for i in range(NCORES)
        ]
    else:
        in_maps = [{"x": _shard_tmajor(x, i)} for i in range(NCORES)]
    res = run_bass_kernel_spmd(
        nc, in_maps, core_ids=list(range(NCORES)), trace=_trace
    )
    invert = _mode() == "pe"                        # pe packs the keep-mask
    outs = []
    for r in res.results:
        ot = r["o"]
        if ot.size == SPAT:                         # bit-packed u8: bit t = o_t
            if invert:
                ot = np.invert(ot)
            bits = np.unpackbits(
                ot.reshape(-1, 1), axis=1, bitorder="little"
            )[:, :T]
            outs.append(bits.reshape(BS, C, H, W, T).astype(np.float32))
            continue
        if _mode() == "h":                          # u8 masks, mixed polarity
            import os

            jm7 = int(os.environ.get("LIF_JM7", "0"))
            mulf = os.environ.get("LIF_MULF", "0") == "1"
            nch = SPAT // (P * FD)
            # rows (t,p), cols (c,j) -> [T, nch, P, FD]
            bits = (ot != 0).reshape(T, P, nch, FD).transpose(0, 2, 1, 3)
            if not mulf:                            # keep-mask blocks invert
                for t in range(T - 1):
                    for c in range(nch):
                        if _h_ismul(t, c, nch, jm7):
                            bits[t, c] = ~bits[t, c]
            ot = bits.reshape(T, SPAT).astype(np.float32)
            outs.append(np.moveaxis(ot, 0, -1).reshape(BS, C, H, W, T))
            continue
        if ot.dtype != np.float32:                  # int8 spikes -> f32
            ot = (ot != 0).astype(np.float32)
        ot = ot.reshape(T, SPAT)                    # time-major back to T-last
        outs.append(np.moveaxis(ot, 0, -1).reshape(BS, C, H, W, T))
    out = np.ascontiguousarray(np.concatenate(outs, axis=0))
    return (out, res) if _trace else out

